# revision 1
# baseline (speedup 1.0000x reference)
"""Trainium2 Bass kernel for nn_CrossAttention (dense_transformer).

Reference computation (per batch b, per stream s in {1,2}):
    q_s   = heads(x_s)                      # [H, N, D] slices of x_s
    kv_s  = x_s @ Wkv_s -> k_s, v_s         # [N, C] each
    gate_s= sigmoid(relu(x_s @ w1 + b1) @ w2 + b2)
    ctx_s = softmax_d( scale * k_s^T @ (v_s * gate_s) )   # [H, D, D], softmax over d
    o_1   = q_1 @ ctx_2 ; o_2 = q_2 @ ctx_1  (cross)

Sharding: 8 cores = (stream s, batch b) pairs.  Core (s, b) projects
x_s[b] (kv + gate + ctx_s[b]) and then computes the OTHER stream's
output o_{1-s}[b] = q_{1-s}[b] @ softmax(ctx_s[b]).  No cross-core
communication; host concatenates outputs.
"""

import numpy as np
from contextlib import ExitStack

N = 4096
C = 1024
H = 16
D = 64
SCALE = D ** (-0.5)
NCH = N // 128       # 32 n-chunks of 128 rows
KCH = C // 128       # 8 contraction chunks
F32 = None           # set lazily (mybir import)

_CACHE = {}


def _build_program(with_bias):
    """Build the SPMD Bass program (same for all 8 cores)."""
    import concourse.bass as bass
    import concourse.bacc as bacc
    import concourse.tile as tile
    import concourse.mybir as mybir

    F32 = mybir.dt.float32
    F32R = mybir.dt.float32r
    BF16 = mybir.dt.bfloat16
    AF = mybir.ActivationFunctionType

    nc = bacc.Bacc("TRN2", target_bir_lowering=False, debug=False, num_devices=8)

    xp = nc.dram_tensor("xp", [N, C], F32R, kind="ExternalInput").ap()
    xq = nc.dram_tensor("xq", [N, C], F32R, kind="ExternalInput").ap()
    wkv = nc.dram_tensor("wkv", [C, 2 * C], F32R, kind="ExternalInput").ap()
    w1 = nc.dram_tensor("w1", [C, C], F32R, kind="ExternalInput").ap()
    b1 = nc.dram_tensor("b1", [C], F32, kind="ExternalInput").ap()
    w2 = nc.dram_tensor("w2", [C, C], F32R, kind="ExternalInput").ap()
    b2 = nc.dram_tensor("b2", [C], F32R, kind="ExternalInput").ap()
    ident = nc.dram_tensor("ident", [128, 128], F32R, kind="ExternalInput").ap()
    identb = nc.dram_tensor("identb", [128, 128], BF16, kind="ExternalInput").ap()
    o = nc.dram_tensor("o", [N, C], F32R, kind="ExternalOutput").ap()


    with tile.TileContext(nc) as tc, ExitStack() as ctx:
        # ---------- persistent pools ----------
        cpool = ctx.enter_context(tc.tile_pool(name="consts", bufs=1))
        ident_sb = cpool.tile([128, 128], F32R, name="ident_sb")
        nc.sync.dma_start(ident_sb, ident)
        identf = cpool.tile([128, 128], F32, name="identf")
        nc.vector.tensor_copy(identf, ident_sb)
        identb_sb = cpool.tile([128, 128], BF16, name="identb_sb")
        nc.sync.dma_start(identb_sb, identb)
        b1_sb = cpool.tile([128, 8], F32, name="b1_sb")  # b1_sb[p, m] = b1[m*128+p]
        nc.sync.dma_start(b1_sb, b1.rearrange("(m p) -> p m", p=128))
        if with_bias:
            ones_sb = cpool.tile([1, 128], F32, name="ones_sb")
            nc.vector.memset(ones_sb, 1.0)
            ones_r = cpool.tile([1, 128], F32R, name="ones_r")
            nc.vector.tensor_copy(ones_r, ones_sb)
            b2_r = cpool.tile([1, C], F32R, name="b2_r")
            nc.sync.dma_start(b2_r, b2.rearrange("(one f) -> one f", one=1))

        acc_pool = ctx.enter_context(tc.tile_pool(name="ctxacc", bufs=1))
        # ctxT accumulator on partitions 0-63: head h -> cols [h*64, h*64+64), layout [e, d]
        ctx_acc = acc_pool.tile([64, 1024], F32, name="ctx_acc")
        nc.vector.memset(ctx_acc, 0.0)

        spool = ctx.enter_context(tc.tile_pool(name="spairs", bufs=1))
        spairs = [spool.tile([128, 128], BF16, name=f"spair{j}") for j in range(8)]

        dpool = ctx.enter_context(tc.tile_pool(name="scratch", bufs=1, space="DRAM"))
        g_dram = dpool.tile([N, C], F32, name="g_dram")
        xpT_dram = dpool.tile([C, N], F32R, name="xpT_dram")

        # =========================================================
        # Phase A1: gate MLP for all n; also builds/spills xp^T.
        #   gate1 transposed-out: hT[m-tile, n] = (xp @ w1).T  (w1 stationary)
        #   gate2 normal-out:     g[n, :] = sigmoid(h @ w2 + b2)  (hT stationary)
        # =========================================================
        with ExitStack() as a1:
            wpool = a1.enter_context(tc.tile_pool(name="a1w", bufs=1))
            w1_sb = wpool.tile([128, 8, C], F32R, name="w1_sb")  # [p, k, col]
            nc.sync.dma_start(w1_sb, w1.rearrange("(k p) m -> p k m", p=128))
            w2_sb = wpool.tile([128, 8, C], F32R, name="w2_sb")
            nc.sync.dma_start(w2_sb, w2.rearrange("(k p) m -> p k m", p=128))

            ht_pool = a1.enter_context(tc.tile_pool(name="a1ht", bufs=1))
            gout_pool = a1.enter_context(tc.tile_pool(name="a1g", bufs=1))
            g1ps_pool = a1.enter_context(
                tc.tile_pool(name="a1g1ps", bufs=4, space="PSUM")
            )
            g2ps_pool = a1.enter_context(
                tc.tile_pool(name="a1g2ps", bufs=2, space="PSUM")
            )

            def emit_transposes_g1(sb, xpt_pool, xin_pool, trps_pool):
                xpt = [
                    xpt_pool.tile([128, 1024], F32R, name=f"xpt{j}", tag=f"xpt{j}", bufs=1)
                    for j in range(8)
                ]
                for grp in range(2):  # 512-row halves
                    xins = []
                    for c4 in range(4):
                        xin = xin_pool.tile([128, C], F32R, name="xin", tag="xin")
                        nch = sb * 8 + grp * 4 + c4
                        nc.sync.dma_start(xin, xp[nch * 128:(nch + 1) * 128, :])
                        xins.append(xin)
                    for j in range(8):
                        tps = trps_pool.tile([128, 512], F32R, name="tps", tag="tps")
                        for c4 in range(4):
                            nc.tensor.transpose(
                                tps[:, c4 * 128:(c4 + 1) * 128],
                                xins[c4][:, j * 128:(j + 1) * 128],
                                ident_sb,
                            )
                        if j % 2 == 0:
                            nc.vector.tensor_copy(
                                xpt[j][:, grp * 512:(grp + 1) * 512], tps
                            )
                        else:
                            nc.scalar.copy(
                                xpt[j][:, grp * 512:(grp + 1) * 512], tps
                            )
                # spill xp^T
                for j in range(8):
                    nc.sync.dma_start(
                        xpT_dram[j * 128:(j + 1) * 128, sb * 1024:(sb + 1) * 1024],
                        xpt[j],
                    )
                # gate1 transposed: hT[m] = sum_k w1[k,m].T @ xpT[k]
                hts = [
                    ht_pool.tile([128, 1024], F32R, name=f"ht{m}", tag=f"ht{m}", bufs=2)
                    for m in range(8)
                ]
                for m in range(8):
                    pss = [
                        g1ps_pool.tile([128, 512], F32, name="g1ps", tag="g1ps")
                        for _ in range(2)
                    ]
                    for k in range(8):
                        lhs = w1_sb[:, k, m * 128:(m + 1) * 128]
                        for half in range(2):
                            nc.tensor.matmul(
                                pss[half],
                                lhs,
                                xpt[k][:, half * 512:(half + 1) * 512],
                                start=(k == 0),
                                stop=(k == 7),
                            )
                    for half in range(2):
                        nc.scalar.activation(
                            hts[m][:, half * 512:(half + 1) * 512],
                            pss[half],
                            AF.Relu,
                            bias=b1_sb[:, m:m + 1],
                        )
                return hts

            def emit_g2(sb, hts):
                for c in range(8):
                    nch = sb * 8 + c
                    gt = gout_pool.tile([128, C], F32, name="gt", tag="gt")
                    for t in range(2):
                        ps2 = g2ps_pool.tile([128, 512], F32, name="g2ps", tag="g2ps")
                        for k in range(8):
                            nc.tensor.matmul(
                                ps2,
                                hts[k][:, c * 128:(c + 1) * 128],
                                w2_sb[:, k, t * 512:(t + 1) * 512],
                                start=(k == 0),
                                stop=(k == 7 and not with_bias),
                            )
                        if with_bias:
                            nc.tensor.matmul(
                                ps2,
                                ones_r,
                                b2_r[:, t * 512:(t + 1) * 512],
                                start=False,
                                stop=True,
                            )
                        nc.scalar.activation(
                            gt[:, t * 512:(t + 1) * 512], ps2, AF.Sigmoid
                        )
                    nc.sync.dma_start(g_dram[nch * 128:(nch + 1) * 128, :], gt)

            with ExitStack() as a1inner:
                xpt_pool_i = a1inner.enter_context(tc.tile_pool(name="a1xpt", bufs=1))
                xin_pool_i = a1inner.enter_context(tc.tile_pool(name="a1xin", bufs=6))
                trps_pool_i = a1inner.enter_context(
                    tc.tile_pool(name="a1trps", bufs=2, space="PSUM")
                )
                for sb in range(3):
                    hts = emit_transposes_g1(sb, xpt_pool_i, xin_pool_i, trps_pool_i)
                    emit_g2(sb, hts)
                hts3 = emit_transposes_g1(3, xpt_pool_i, xin_pool_i, trps_pool_i)
            # xpt/xin/trps pools are now closed: A2's wkv tile will alias their
            # space, so its DMA can start while gate2(sb3) still runs on PE.
            emit_g2(3, hts3)

        # early phase-B pools: transposing xq is independent of A2/SM, so give
        # it non-aliased space and let the scheduler overlap it with A2/SM.
        bxin_pool = ctx.enter_context(tc.tile_pool(name="bxin", bufs=5))
        bxqt_pool = ctx.enter_context(tc.tile_pool(name="bxqt", bufs=1))
        btrps_early_pool = ctx.enter_context(
            tc.tile_pool(name="btrpse", bufs=2, space="PSUM")
        )
        bxqt_tiles = {}

        def emit_xq_transposes(blk):
            xins = []
            for c4 in range(4):
                xin = bxin_pool.tile([128, C], F32R, name="bxin", tag="bxin")
                nch = blk * 4 + c4
                nc.sync.dma_start(xin, xq[nch * 128:(nch + 1) * 128, :])
                xinb = bxin_pool.tile([128, C], BF16, name="bxinb", tag="bxinb")
                if c4 % 2 == 0:
                    nc.vector.tensor_copy(xinb, xin)
                else:
                    nc.scalar.copy(xinb, xin)
                xins.append(xinb)
            xqts = [
                bxqt_pool.tile(
                    [128, 512], BF16, name=f"xqt{j}", tag=f"xqt{j}", bufs=3
                )
                for j in range(8)
            ]
            for j in range(8):
                tps = btrps_early_pool.tile(
                    [128, 512], BF16, name="btps", tag="btps"
                )
                for c4 in range(4):
                    nc.tensor.transpose(
                        tps[:, c4 * 128:(c4 + 1) * 128],
                        xins[c4][:, j * 128:(j + 1) * 128],
                        identb_sb,
                    )
                if j % 2 == 0:
                    nc.vector.tensor_copy(xqts[j], tps)
                else:
                    nc.scalar.copy(xqts[j], tps)
            bxqt_tiles[blk] = xqts

        emit_xq_transposes(0)
        emit_xq_transposes(1)
        emit_xq_transposes(2)

        # =========================================================
        # Phase A2: kv projection + ctx accumulation.
        #   kv normal-out (xpT stationary); ctxT_h += vg_h.T @ k_h
        # =========================================================
        with ExitStack() as a2:
            wkv_pool = a2.enter_context(tc.tile_pool(name="a2w", bufs=1))
            wkv_sb = wkv_pool.tile([128, 8, 2 * C], F32R, name="wkv_sb")
            nc.sync.dma_start(wkv_sb, wkv.rearrange("(k p) m -> p k m", p=128))

            xpt_in_pool = a2.enter_context(tc.tile_pool(name="a2xpt", bufs=3))
            gin_pool = a2.enter_context(tc.tile_pool(name="a2gin", bufs=3))
            k_pool = a2.enter_context(tc.tile_pool(name="a2k", bufs=2))
            v_pool = a2.enter_context(tc.tile_pool(name="a2v", bufs=2))
            vg_pool = a2.enter_context(tc.tile_pool(name="a2vg", bufs=2))
            kvps_pool = a2.enter_context(
                tc.tile_pool(name="a2kvps", bufs=4, space="PSUM")
            )
            ctps_pool = a2.enter_context(
                tc.tile_pool(name="a2ctps", bufs=1, space="PSUM")
            )

            for nch in range(NCH):
                xpt_in = xpt_in_pool.tile([128, C], F32R, name="xpt_in", tag="xpt_in")
                nc.sync.dma_start(
                    xpt_in,
                    xpT_dram.rearrange("(k p) n -> p k n", p=128)[
                        :, :, nch * 128:(nch + 1) * 128
                    ],
                )
                gin = gin_pool.tile([128, C], F32, name="gin", tag="gin")
                nc.sync.dma_start(gin, g_dram[nch * 128:(nch + 1) * 128, :])

                kvps = [
                    kvps_pool.tile([128, 512], F32, name="kvps", tag="kvps")
                    for _ in range(4)
                ]
                for k in range(8):
                    lhs = xpt_in[:, k * 128:(k + 1) * 128]
                    for t in range(4):
                        nc.tensor.matmul(
                            kvps[t],
                            lhs,
                            wkv_sb[:, k, t * 512:(t + 1) * 512],
                            start=(k == 0),
                            stop=(k == 7),
                        )
                k_sb = k_pool.tile([128, C], F32R, name="k_sb", tag="k_sb")
                v_sb = v_pool.tile([128, C], F32, name="v_sb", tag="v_sb")
                nc.scalar.copy(k_sb[:, 0:512], kvps[0])
                nc.scalar.copy(k_sb[:, 512:1024], kvps[1])
                nc.vector.tensor_copy(v_sb[:, 0:512], kvps[2])
                nc.vector.tensor_copy(v_sb[:, 512:1024], kvps[3])
                vg = vg_pool.tile([128, C], F32R, name="vg", tag="vg")
                nc.vector.tensor_mul(vg, v_sb, gin)

                ctp = ctps_pool.tile([64, 1024], F32, name="ctp", tag="ctp")
                for h in range(H):
                    nc.tensor.matmul(
                        ctp[:, h * D:(h + 1) * D],
                        vg[:, h * D:(h + 1) * D],
                        k_sb[:, h * D:(h + 1) * D],
                        start=True,
                        stop=True,
                        skip_group_check=True,
                    )
                nc.vector.tensor_add(ctx_acc, ctx_acc, ctp)

        # =========================================================
        # Softmax over d (free dim of ctxT) + build block-diag S pairs
        # =========================================================
        with ExitStack() as sm:
            smp = sm.enter_context(tc.tile_pool(name="smpool", bufs=1))
            smps = sm.enter_context(tc.tile_pool(name="smps", bufs=2, space="PSUM"))
            maxs = smp.tile([64, 16], F32, name="maxs")
            nc.vector.tensor_reduce(
                maxs,
                ctx_acc.rearrange("p (b d) -> p b d", b=16),
                axis=mybir.AxisListType.X,
                op=mybir.AluOpType.max,
            )
            cmx = smp.tile([64, 1024], F32, name="cmx")
            nc.vector.tensor_sub(
                cmx.rearrange("p (h d) -> p h d", h=16),
                ctx_acc.rearrange("p (h d) -> p h d", h=16),
                maxs.unsqueeze(-1).broadcast_to([64, 16, 64]),
            )
            et = smp.tile([64, 1024], F32, name="et")
            nc.scalar.activation(et, cmx, AF.Exp, scale=float(SCALE))
            sums = smp.tile([64, 16], F32, name="sums")
            nc.vector.tensor_reduce(
                sums,
                et.rearrange("p (b d) -> p b d", b=16),
                axis=mybir.AxisListType.X,
                op=mybir.AluOpType.add,
            )
            recs = smp.tile([64, 16], F32, name="recs")
            nc.vector.reciprocal(recs, sums)
            st = smp.tile([64, 1024], F32, name="st")
            nc.vector.tensor_mul(
                st.rearrange("p (h d) -> p h d", h=16),
                et.rearrange("p (h d) -> p h d", h=16),
                recs.unsqueeze(-1).broadcast_to([64, 16, 64]),
            )
            # st: softmaxed ctxT [e, d] per head at cols h*64.  Transposing the
            # side-by-side pair [ctxT_2j | ctxT_2j+1] ([64, 128]) gives
            # [S_2j stacked above S_2j+1] ([128, 64]); scatter to block-diag.
            zero_sb = smp.tile([128, 128], BF16, name="zero_sb")
            nc.vector.memset(zero_sb, 0.0)
            for j in range(8):
                tp = smps.tile([128, 64], F32, name="smtp", tag="smtp")
                nc.tensor.transpose(
                    tp, st[:, (2 * j) * 64:(2 * j + 2) * 64], identf[0:64, 0:64]
                )
                nc.vector.tensor_copy(spairs[j], zero_sb)
                nc.vector.tensor_copy(spairs[j][0:64, 0:64], tp[0:64, :])
                nc.vector.tensor_copy(spairs[j][64:128, 64:128], tp[64:128, :])

        # =========================================================
        # Phase B: o[nchunk, j*128:(j+1)*128] = (xqT_j_chunk).T @ spair_j
        # (normal orientation directly; no back-transposes)
        # =========================================================
        with ExitStack() as pb:
            oout_pool = pb.enter_context(tc.tile_pool(name="bo", bufs=6))
            bops_pool = pb.enter_context(tc.tile_pool(name="bops", bufs=4, space="PSUM"))

            for blk in range(8):
                if blk + 3 < 8:
                    emit_xq_transposes(blk + 3)
                xqts = bxqt_tiles.pop(blk)
                oouts = [
                    oout_pool.tile([128, C], F32R, name="oo", tag="oo")
                    for _ in range(4)
                ]
                for c4 in range(4):
                    for half in range(2):
                        ops = bops_pool.tile([128, 512], F32, name="ops", tag="ops")
                        for jj in range(4):
                            j = half * 4 + jj
                            nc.tensor.matmul(
                                ops[:, jj * 128:(jj + 1) * 128],
                                xqts[j][:, c4 * 128:(c4 + 1) * 128],
                                spairs[j],
                                start=True,
                                stop=True,
                                skip_group_check=True,
                            )
                        if half == 0:
                            nc.vector.tensor_copy(
                                oouts[c4][:, half * 512:(half + 1) * 512], ops
                            )
                        else:
                            nc.scalar.copy(
                                oouts[c4][:, half * 512:(half + 1) * 512], ops
                            )
                for c4 in range(4):
                    nch = blk * 4 + c4
                    nc.sync.dma_start(o[nch * 128:(nch + 1) * 128, :], oouts[c4])

    nc.compile()
    return nc


def _get_program(with_bias=False):
    key = ("nc", bool(with_bias))
    if key not in _CACHE:
        _CACHE[key] = _build_program(with_bias)
    return _CACHE[key]


def make_in_maps(x1, x2, Wkv1, Wkv2, g1_w1, g1_b1, g1_w2, g1_b2,
                 g2_w1, g2_b1, g2_w2, g2_b2):
    """Core (s, b): cores 0-3 = (s=0, b), cores 4-7 = (s=1, b)."""
    import ml_dtypes
    ident = np.eye(128, dtype=np.float32)
    identb = np.eye(128, dtype=ml_dtypes.bfloat16)
    asf = np.ascontiguousarray
    in_maps = []
    for core in range(8):
        s, b = core // 4, core % 4
        if s == 0:
            m = dict(xp=asf(x1[b]), xq=asf(x2[b]), wkv=asf(Wkv1),
                     w1=asf(g1_w1), b1=asf(g1_b1), w2=asf(g1_w2), b2=asf(g1_b2))
        else:
            m = dict(xp=asf(x2[b]), xq=asf(x1[b]), wkv=asf(Wkv2),
                     w1=asf(g2_w1), b1=asf(g2_b1), w2=asf(g2_w2), b2=asf(g2_b2))
        m["ident"] = ident
        m["identb"] = identb
        in_maps.append(m)
    return in_maps


def kernel(x1, x2, Wkv1, Wkv2, g1_w1, g1_b1, g1_w2, g1_b2,
           g2_w1, g2_b1, g2_w2, g2_b2, _runner=None):
    """Full-input entry point.  Returns (o1, o2), each [4, 4096, 1024] f32."""
    from concourse.bass_utils import run_bass_kernel_spmd

    args = [np.asarray(a, dtype=np.float32) for a in
            (x1, x2, Wkv1, Wkv2, g1_w1, g1_b1, g1_w2, g1_b2,
             g2_w1, g2_b1, g2_w2, g2_b2)]
    with_bias = bool(np.any(args[7]) or np.any(args[11]))  # g1_b2, g2_b2
    nc = _get_program(with_bias)
    in_maps = make_in_maps(*args)
    if _runner is None:
        res = run_bass_kernel_spmd(nc, in_maps, core_ids=list(range(8)))
        results = res.results
    else:
        results = _runner(nc, in_maps)

    B = x1.shape[0]
    o1 = np.empty((B, N, C), dtype=np.float32)
    o2 = np.empty((B, N, C), dtype=np.float32)
    for core in range(8):
        s, b = core // 4, core % 4
        out = results[core]["o"]
        if s == 0:
            o2[b] = out   # core projected x1 -> ctx1 -> o2 = q2 @ ctx1
        else:
            o1[b] = out
    return (o1, o2)



# revision 3
# speedup vs baseline: 1.2438x; 1.2438x over previous
"""Trainium2 Bass kernel for nn_CrossAttention (dense_transformer).

Reference computation (per batch b, per stream s in {1,2}):
    q_s   = heads(x_s)                      # [H, N, D] slices of x_s
    kv_s  = x_s @ Wkv_s -> k_s, v_s         # [N, C] each
    gate_s= sigmoid(relu(x_s @ w1 + b1) @ w2 + b2)
    ctx_s = softmax_d( scale * k_s^T @ (v_s * gate_s) )   # [H, D, D], softmax over d
    o_1   = q_1 @ ctx_2 ; o_2 = q_2 @ ctx_1  (cross)

Sharding: 8 cores = (stream s, batch b) pairs.  Core (s, b) projects
x_s[b] (kv + gate + ctx_s[b]) and then computes the OTHER stream's
output o_{1-s}[b] = q_{1-s}[b] @ softmax(ctx_s[b]).  No cross-core
communication; host concatenates outputs.

This version: fully-fused single streaming pass (no DRAM spills).
All matmul operands are bf16 (converted on host); PSUM-resident ctx
accumulation; output written bf16 and upcast on host.
"""

import numpy as np
from contextlib import ExitStack

N = 4096
C = 1024
H = 16
D = 64
SCALE = D ** (-0.5)
R = 512              # rows per A-phase block
NBLK = N // R        # 8 blocks
KCH = C // 128       # 8 contraction chunks

_CACHE = {}


def _build_program(with_bias):
    """Build the SPMD Bass program (same for all 8 cores)."""
    import concourse.bass as bass
    import concourse.bacc as bacc
    import concourse.tile as tile
    import concourse.mybir as mybir

    F32 = mybir.dt.float32
    BF16 = mybir.dt.bfloat16
    AF = mybir.ActivationFunctionType

    nc = bacc.Bacc("TRN2", target_bir_lowering=False, debug=False, num_devices=8)

    xp = nc.dram_tensor("xp", [N, C], BF16, kind="ExternalInput").ap()
    xq = nc.dram_tensor("xq", [N, C], BF16, kind="ExternalInput").ap()
    wkv = nc.dram_tensor("wkv", [C, 2 * C], BF16, kind="ExternalInput").ap()
    w1 = nc.dram_tensor("w1", [C, C], BF16, kind="ExternalInput").ap()
    b1 = nc.dram_tensor("b1", [C], F32, kind="ExternalInput").ap()
    w2 = nc.dram_tensor("w2", [C, C], BF16, kind="ExternalInput").ap()
    b2 = nc.dram_tensor("b2", [C], BF16, kind="ExternalInput").ap()
    identb = nc.dram_tensor("identb", [128, 128], BF16, kind="ExternalInput").ap()
    ident64 = nc.dram_tensor("ident64", [64, 64], F32, kind="ExternalInput").ap()
    o = nc.dram_tensor("o", [N, C], BF16, kind="ExternalOutput").ap()

    with tile.TileContext(nc) as tc, ExitStack() as ctx:
        # ---------- persistent pools ----------
        cpool = ctx.enter_context(tc.tile_pool(name="consts", bufs=1))
        identb_sb = cpool.tile([128, 128], BF16, name="identb_sb")
        nc.sync.dma_start(identb_sb, identb)
        ident64_sb = cpool.tile([64, 64], F32, name="ident64_sb")
        nc.sync.dma_start(ident64_sb, ident64)
        b1_sb = cpool.tile([128, 8], F32, name="b1_sb")  # b1_sb[p, m] = b1[m*128+p]
        nc.sync.dma_start(b1_sb, b1.rearrange("(m p) -> p m", p=128))
        if with_bias:
            ones_b = cpool.tile([1, 128], BF16, name="ones_b")
            nc.vector.memset(ones_b, 1.0)
            b2_r = cpool.tile([1, C], BF16, name="b2_r")
            nc.sync.dma_start(b2_r, b2.rearrange("(one f) -> one f", one=1))

        wpool = ctx.enter_context(tc.tile_pool(name="weights", bufs=1))
        w1_sb = wpool.tile([128, 8, C], BF16, name="w1_sb")  # [p, k, m]
        nc.sync.dma_start(w1_sb, w1.rearrange("(k p) m -> p k m", p=128))
        w2_sb = wpool.tile([128, 8, C], BF16, name="w2_sb")
        nc.sync.dma_start(w2_sb, w2.rearrange("(k p) m -> p k m", p=128))
        wkv_sb = wpool.tile([128, 8, 2 * C], BF16, name="wkv_sb")
        nc.sync.dma_start(wkv_sb, wkv.rearrange("(k p) m -> p k m", p=128))

        # ctx accumulators: ctxT per head h at cols (h%8)*64, layout [e, d]
        # heads 0-7 in ctx_ps[0], heads 8-15 in ctx_ps[1]; PSUM-resident.
        ctxp_pool = ctx.enter_context(tc.tile_pool(name="ctxps", bufs=1, space="PSUM"))
        ctx_ps = [
            ctxp_pool.tile([64, 512], F32, name=f"ctx_ps{i}") for i in range(2)
        ]

        # shared transient PSUM pool (transposes, gate1/2, kv, phase B)
        mmps_pool = ctx.enter_context(tc.tile_pool(name="mmps", bufs=6, space="PSUM"))

        def mmps(shape, dtype):
            return mmps_pool.tile(shape, dtype, name="mmps", tag="mmps")

        spool = ctx.enter_context(tc.tile_pool(name="spairs", bufs=1))
        spairs = [spool.tile([128, 128], BF16, name=f"spair{j}") for j in range(8)]

        # ---------- streaming pools ----------
        xpin_pool = ctx.enter_context(tc.tile_pool(name="xpin", bufs=5))
        xqin_pool = ctx.enter_context(tc.tile_pool(name="xqin", bufs=6))
        xpT_pool = ctx.enter_context(tc.tile_pool(name="xpT", bufs=2))
        hT_pool = ctx.enter_context(tc.tile_pool(name="hT", bufs=2))
        g_pool = ctx.enter_context(tc.tile_pool(name="g", bufs=4))
        k_pool = ctx.enter_context(tc.tile_pool(name="k", bufs=3))
        vg_pool = ctx.enter_context(tc.tile_pool(name="vg", bufs=3))
        xqT_pool = ctx.enter_context(tc.tile_pool(name="xqT", bufs=4))
        oout_pool = ctx.enter_context(tc.tile_pool(name="oout", bufs=3))

        xqT_tiles = {}

        def emit_xq_trans(blk):
            """Transpose xq rows [blk*512, (blk+1)*512) -> xqT [128, j, 512]."""
            xins = []
            for c4 in range(4):
                xin = xqin_pool.tile([128, C], BF16, name="xqin", tag="xqin")
                nch = blk * 4 + c4
                nc.sync.dma_start(xin, xq[nch * 128:(nch + 1) * 128, :])
                xins.append(xin)
            xqT = xqT_pool.tile([128, 8, R], BF16, name="xqT", tag="xqT")
            for j in range(8):
                trp = mmps([128, R], BF16)
                for c4 in range(4):
                    nc.tensor.transpose(
                        trp[:, c4 * 128:(c4 + 1) * 128],
                        xins[c4][:, j * 128:(j + 1) * 128],
                        identb_sb,
                    )
                if j % 2 == 0:
                    nc.vector.tensor_copy(xqT[:, j, :], trp)
                else:
                    nc.scalar.copy(xqT[:, j, :], trp)
            xqT_tiles[blk] = xqT

        # early xq transposes cover the w1/w2 weight-load latency
        emit_xq_trans(0)
        emit_xq_trans(1)

        # =========================================================
        # Phase A: per 512-row block: transpose -> gate1 -> gate2 ->
        #          kv -> ctx accumulation (PSUM)
        # =========================================================
        for blk in range(NBLK):
            # ---- load + transpose xp block ----
            xins = []
            for c4 in range(4):
                xin = xpin_pool.tile([128, C], BF16, name="xpin", tag="xpin")
                nch = blk * 4 + c4
                nc.sync.dma_start(xin, xp[nch * 128:(nch + 1) * 128, :])
                xins.append(xin)
            xpT = xpT_pool.tile([128, 8, R], BF16, name="xpT", tag="xpT")
            for j in range(8):
                trp = mmps([128, R], BF16)
                for c4 in range(4):
                    nc.tensor.transpose(
                        trp[:, c4 * 128:(c4 + 1) * 128],
                        xins[c4][:, j * 128:(j + 1) * 128],
                        identb_sb,
                    )
                if j % 2 == 0:
                    nc.vector.tensor_copy(xpT[:, j, :], trp)
                else:
                    nc.scalar.copy(xpT[:, j, :], trp)

            # ---- gate1: hT[m, n] = relu(b1 + sum_k w1[k,m]^T xpT[k]) ----
            hT = hT_pool.tile([128, 8, R], BF16, name="hT", tag="hT")
            for m in range(8):
                ps = mmps([128, R], F32)
                for k in range(8):
                    nc.tensor.matmul(
                        ps,
                        w1_sb[:, k, m * 128:(m + 1) * 128],
                        xpT[:, k, :],
                        start=(k == 0),
                        stop=(k == 7),
                    )
                nc.scalar.activation(
                    hT[:, m, :], ps, AF.Relu, bias=b1_sb[:, m:m + 1]
                )

            # ---- gate2: g[n, :] = sigmoid(sum_k hT[k]^T w2[k] + b2) ----
            gts = []
            for c4 in range(4):
                gt = g_pool.tile([128, C], BF16, name="gt", tag="gt")
                for t in range(2):
                    ps = mmps([128, 512], F32)
                    for k in range(8):
                        nc.tensor.matmul(
                            ps,
                            hT[:, k, c4 * 128:(c4 + 1) * 128],
                            w2_sb[:, k, t * 512:(t + 1) * 512],
                            start=(k == 0),
                            stop=(k == 7 and not with_bias),
                        )
                    if with_bias:
                        nc.tensor.matmul(
                            ps,
                            ones_b,
                            b2_r[:, t * 512:(t + 1) * 512],
                            start=False,
                            stop=True,
                        )
                    nc.scalar.activation(
                        gt[:, t * 512:(t + 1) * 512], ps, AF.Sigmoid
                    )
                gts.append(gt)

            # ---- kv projection + ctx accumulation, per 128-row chunk ----
            def emit_kv(c4):
                k_bf = k_pool.tile([128, C], BF16, name="k_bf", tag="k_bf")
                vg = vg_pool.tile([128, C], BF16, name="vg", tag="vg")
                for t in range(4):
                    ps = mmps([128, 512], F32)
                    for k in range(8):
                        nc.tensor.matmul(
                            ps,
                            xpT[:, k, c4 * 128:(c4 + 1) * 128],
                            wkv_sb[:, k, t * 512:(t + 1) * 512],
                            start=(k == 0),
                            stop=(k == 7),
                        )
                    if t < 2:
                        nc.scalar.copy(k_bf[:, t * 512:(t + 1) * 512], ps)
                    else:
                        nc.vector.tensor_mul(
                            vg[:, (t - 2) * 512:(t - 1) * 512],
                            ps,
                            gts[c4][:, (t - 2) * 512:(t - 1) * 512],
                        )
                return k_bf, vg

            def emit_ctx(c4, kv_tiles):
                k_bf, vg = kv_tiles
                first = (blk == 0 and c4 == 0)
                last = (blk == NBLK - 1 and c4 == 3)
                for h in range(H):
                    hp, hc = h // 8, h % 8
                    # start=True clears has_written for the WHOLE bank, so only
                    # the very first matmul into each bank may set it.
                    nc.tensor.matmul(
                        ctx_ps[hp][:, hc * 64:(hc + 1) * 64],
                        vg[:, h * D:(h + 1) * D],
                        k_bf[:, h * D:(h + 1) * D],
                        start=(first and hc == 0),
                        stop=last,
                        skip_group_check=True,
                    )

            kvt = {}
            kvt[0] = emit_kv(0)
            kvt[1] = emit_kv(1)
            emit_ctx(0, kvt[0])
            kvt[2] = emit_kv(2)
            emit_ctx(1, kvt[1])
            kvt[3] = emit_kv(3)
            emit_ctx(2, kvt[2])
            emit_ctx(3, kvt[3])

        # =========================================================
        # Softmax over d (free dim of ctxT) + build block-diag S pairs
        # =========================================================
        with ExitStack() as sm:
            smp = sm.enter_context(tc.tile_pool(name="smpool", bufs=1))
            maxs = smp.tile([64, 16], F32, name="maxs")
            sums = smp.tile([64, 16], F32, name="sums")
            for i in range(2):
                nc.vector.tensor_reduce(
                    maxs[:, i * 8:(i + 1) * 8],
                    ctx_ps[i].rearrange("p (b d) -> p b d", b=8),
                    axis=mybir.AxisListType.X,
                    op=mybir.AluOpType.max,
                )
            cmx = smp.tile([64, 1024], F32, name="cmx")
            for i in range(2):
                nc.vector.tensor_sub(
                    cmx.rearrange("p (h d) -> p h d", h=16)[:, i * 8:(i + 1) * 8, :],
                    ctx_ps[i].rearrange("p (h d) -> p h d", h=8),
                    maxs[:, i * 8:(i + 1) * 8].unsqueeze(-1).broadcast_to(
                        [64, 8, 64]
                    ),
                )
            et = smp.tile([64, 1024], F32, name="et")
            nc.scalar.activation(et, cmx, AF.Exp, scale=float(SCALE))
            nc.vector.tensor_reduce(
                sums,
                et.rearrange("p (b d) -> p b d", b=16),
                axis=mybir.AxisListType.X,
                op=mybir.AluOpType.add,
            )
            recs = smp.tile([64, 16], F32, name="recs")
            nc.vector.reciprocal(recs, sums)
            st = smp.tile([64, 1024], F32, name="st")
            nc.vector.tensor_mul(
                st.rearrange("p (h d) -> p h d", h=16),
                et.rearrange("p (h d) -> p h d", h=16),
                recs.unsqueeze(-1).broadcast_to([64, 16, 64]),
            )
            # st: softmaxed ctxT [e, d] per head at cols h*64.  Transposing the
            # side-by-side pair [ctxT_2j | ctxT_2j+1] ([64, 128]) gives
            # [S_2j stacked above S_2j+1] ([128, 64]); scatter to block-diag.
            zero_sb = smp.tile([128, 128], BF16, name="zero_sb")
            nc.vector.memset(zero_sb, 0.0)
            for j in range(8):
                tp = mmps([128, 64], F32)
                nc.tensor.transpose(
                    tp, st[:, (2 * j) * 64:(2 * j + 2) * 64], ident64_sb
                )
                nc.vector.tensor_copy(spairs[j], zero_sb)
                nc.vector.tensor_copy(spairs[j][0:64, 0:64], tp[0:64, :])
                nc.vector.tensor_copy(spairs[j][64:128, 64:128], tp[64:128, :])

        # =========================================================
        # Phase B: o[nchunk, j*128:(j+1)*128] = (xqT_j_chunk).T @ spair_j
        # =========================================================
        emit_xq_trans(2)
        emit_xq_trans(3)

        def emit_b(blk):
            xqT = xqT_tiles.pop(blk)
            oouts = []
            for c4 in range(4):
                oout = oout_pool.tile([128, C], BF16, name="oo", tag="oo")
                for half in range(2):
                    ps = mmps([128, 512], F32)
                    for jj in range(4):
                        j = half * 4 + jj
                        nc.tensor.matmul(
                            ps[:, jj * 128:(jj + 1) * 128],
                            xqT[:, j, c4 * 128:(c4 + 1) * 128],
                            spairs[j],
                            start=True,
                            stop=True,
                            skip_group_check=True,
                        )
                    if half == 0:
                        nc.vector.tensor_copy(oout[:, 0:512], ps)
                    else:
                        nc.scalar.copy(oout[:, 512:1024], ps)
                oouts.append(oout)
            for c4 in range(4):
                nch = blk * 4 + c4
                nc.sync.dma_start(o[nch * 128:(nch + 1) * 128, :], oouts[c4])

        emit_b(0)
        emit_xq_trans(4)
        emit_b(1)
        emit_xq_trans(5)
        emit_b(2)
        emit_xq_trans(6)
        emit_b(3)
        emit_xq_trans(7)
        emit_b(4)
        emit_b(5)
        emit_b(6)
        emit_b(7)

    nc.compile()
    return nc


def _get_program(with_bias=False):
    key = ("nc", bool(with_bias))
    if key not in _CACHE:
        _CACHE[key] = _build_program(with_bias)
    return _CACHE[key]


def make_in_maps(x1, x2, Wkv1, Wkv2, g1_w1, g1_b1, g1_w2, g1_b2,
                 g2_w1, g2_b1, g2_w2, g2_b2):
    """Core (s, b): cores 0-3 = (s=0, b), cores 4-7 = (s=1, b)."""
    import ml_dtypes
    BF = ml_dtypes.bfloat16
    identb = np.eye(128, dtype=BF)
    ident64 = np.eye(64, dtype=np.float32)

    def bf(a):
        return np.ascontiguousarray(np.asarray(a, np.float32).astype(BF))

    x1b = [bf(x1[b]) for b in range(x1.shape[0])]
    x2b = [bf(x2[b]) for b in range(x2.shape[0])]
    Ws = [
        dict(wkv=bf(Wkv1), w1=bf(g1_w1), b1=np.asarray(g1_b1, np.float32),
             w2=bf(g1_w2), b2=bf(g1_b2)),
        dict(wkv=bf(Wkv2), w1=bf(g2_w1), b1=np.asarray(g2_b1, np.float32),
             w2=bf(g2_w2), b2=bf(g2_b2)),
    ]
    in_maps = []
    for core in range(8):
        s, b = core // 4, core % 4
        m = dict(Ws[s])
        m["xp"] = x1b[b] if s == 0 else x2b[b]
        m["xq"] = x2b[b] if s == 0 else x1b[b]
        m["identb"] = identb
        m["ident64"] = ident64
        in_maps.append(m)
    return in_maps


def kernel(x1, x2, Wkv1, Wkv2, g1_w1, g1_b1, g1_w2, g1_b2,
           g2_w1, g2_b1, g2_w2, g2_b2, _runner=None):
    """Full-input entry point.  Returns (o1, o2), each [4, 4096, 1024] f32."""
    from concourse.bass_utils import run_bass_kernel_spmd

    args = [np.asarray(a, dtype=np.float32) for a in
            (x1, x2, Wkv1, Wkv2, g1_w1, g1_b1, g1_w2, g1_b2,
             g2_w1, g2_b1, g2_w2, g2_b2)]
    with_bias = bool(np.any(args[7]) or np.any(args[11]))  # g1_b2, g2_b2
    nc = _get_program(with_bias)
    in_maps = make_in_maps(*args)
    if _runner is None:
        res = run_bass_kernel_spmd(nc, in_maps, core_ids=list(range(8)))
        results = res.results
    else:
        results = _runner(nc, in_maps)

    B = x1.shape[0]
    o1 = np.empty((B, N, C), dtype=np.float32)
    o2 = np.empty((B, N, C), dtype=np.float32)
    for core in range(8):
        s, b = core // 4, core % 4
        out = np.asarray(results[core]["o"], dtype=np.float32)
        if s == 0:
            o2[b] = out   # core projected x1 -> ctx1 -> o2 = q2 @ ctx1
        else:
            o1[b] = out
    return (o1, o2)


# revision 10
# speedup vs baseline: 1.2991x; 1.0445x over previous
"""Trainium2 Bass kernel for nn_CrossAttention (dense_transformer).

Reference computation (per batch b, per stream s in {1,2}):
    q_s   = heads(x_s)                      # [H, N, D] slices of x_s
    kv_s  = x_s @ Wkv_s -> k_s, v_s         # [N, C] each
    gate_s= sigmoid(relu(x_s @ w1 + b1) @ w2 + b2)
    ctx_s = softmax_d( scale * k_s^T @ (v_s * gate_s) )   # [H, D, D], softmax over d
    o_1   = q_1 @ ctx_2 ; o_2 = q_2 @ ctx_1  (cross)

Sharding: 8 cores = (stream s, batch b) pairs.  Core (s, b) projects
x_s[b] (kv + gate + ctx_s[b]) and then computes the OTHER stream's
output o_{1-s}[b] = q_{1-s}[b] @ softmax(ctx_s[b]).  No cross-core
communication; host concatenates outputs.

Fully-fused single streaming pass (no DRAM spills).  All matmul
operands bf16 (host-converted); PSUM-resident ctx accumulation;
output written bf16 and upcast on host.
"""

import numpy as np
from contextlib import ExitStack

N = 4096
C = 1024
H = 16
D = 64
SCALE = D ** (-0.5)
R = 512              # rows per A-phase block
NBLK = N // R        # 8 blocks
KCH = C // 128       # 8 contraction chunks

_CACHE = {}


def _build_program(with_bias):
    """Build the SPMD Bass program (same for all 8 cores)."""
    import concourse.bass as bass
    import concourse.bacc as bacc
    import concourse.tile as tile
    import concourse.mybir as mybir

    F32 = mybir.dt.float32
    BF16 = mybir.dt.bfloat16
    AF = mybir.ActivationFunctionType

    nc = bacc.Bacc("TRN2", target_bir_lowering=False, debug=False, num_devices=8)

    xp = nc.dram_tensor("xp", [N, C], BF16, kind="ExternalInput").ap()
    xq = nc.dram_tensor("xq", [N, C], BF16, kind="ExternalInput").ap()
    wkv = nc.dram_tensor("wkv", [C, 2 * C], BF16, kind="ExternalInput").ap()
    w1 = nc.dram_tensor("w1", [C, C], BF16, kind="ExternalInput").ap()
    b1 = nc.dram_tensor("b1", [C], F32, kind="ExternalInput").ap()
    w2 = nc.dram_tensor("w2", [C, C], BF16, kind="ExternalInput").ap()
    b2 = nc.dram_tensor("b2", [C], BF16, kind="ExternalInput").ap()
    identb = nc.dram_tensor("identb", [128, 128], BF16, kind="ExternalInput").ap()
    ident64 = nc.dram_tensor("ident64", [64, 64], F32, kind="ExternalInput").ap()
    o = nc.dram_tensor("o", [N, C], BF16, kind="ExternalOutput").ap()

    with tile.TileContext(nc) as tc, ExitStack() as ctx:
        # ---------- pools ----------
        cpool = ctx.enter_context(tc.tile_pool(name="consts", bufs=1))
        wpool = ctx.enter_context(tc.tile_pool(name="weights", bufs=1))
        ctxp_pool = ctx.enter_context(tc.tile_pool(name="ctxps", bufs=1, space="PSUM"))
        mmps_pool = ctx.enter_context(tc.tile_pool(name="mmps", bufs=6, space="PSUM"))
        spool = ctx.enter_context(tc.tile_pool(name="spairs", bufs=1))
        xpin_pool = ctx.enter_context(tc.tile_pool(name="xpin", bufs=2))
        xqin_pool = ctx.enter_context(tc.tile_pool(name="xqin", bufs=2))
        xpT_pool = ctx.enter_context(tc.tile_pool(name="xpT", bufs=1))
        hT_pool = ctx.enter_context(tc.tile_pool(name="hT", bufs=1))
        g_pool = ctx.enter_context(tc.tile_pool(name="g", bufs=4))
        k_pool = ctx.enter_context(tc.tile_pool(name="k", bufs=2))
        vg_pool = ctx.enter_context(tc.tile_pool(name="vg", bufs=2))
        xqT_pool = ctx.enter_context(tc.tile_pool(name="xqT", bufs=6))
        oout_pool = ctx.enter_context(tc.tile_pool(name="oout", bufs=2))

        def mmps(shape, dtype):
            return mmps_pool.tile(shape, dtype, name="mmps", tag="mmps")

        # ---------- DMA priority order: consts, first x blocks, w1 ----------
        identb_sb = cpool.tile([128, 128], BF16, name="identb_sb")
        nc.sync.dma_start(identb_sb, identb)
        ident64_sb = cpool.tile([64, 64], F32, name="ident64_sb")
        nc.sync.dma_start(ident64_sb, ident64)
        b1_sb = cpool.tile([128, 8], F32, name="b1_sb")  # b1_sb[p, m] = b1[m*128+p]
        nc.sync.dma_start(b1_sb, b1.rearrange("(m p) -> p m", p=128))

        def x_dma(pool, src, blk, tag):
            t = pool.tile([128, 4, C], BF16, name=tag, tag=tag)
            nc.sync.dma_start(
                t, src[blk * R:(blk + 1) * R, :].rearrange("(c p) m -> p c m", p=128)
            )
            return t

        xq_tiles = {0: x_dma(xqin_pool, xq, 0, "xqin")}
        xp_tiles = {0: x_dma(xpin_pool, xp, 0, "xpin")}

        w1_sb = wpool.tile([128, 8, C], BF16, name="w1_sb")  # [p, k, m]
        w1r = w1.rearrange("(k p) m -> p k m", p=128)
        for m in range(8):  # split so gate1 m-tile 0 can start early
            nc.sync.dma_start(
                w1_sb[:, :, m * 128:(m + 1) * 128], w1r[:, :, m * 128:(m + 1) * 128]
            )
        xq_tiles[1] = x_dma(xqin_pool, xq, 1, "xqin")
        xp_tiles[1] = x_dma(xpin_pool, xp, 1, "xpin")
        w2_sb = wpool.tile([128, 8, C], BF16, name="w2_sb")
        nc.sync.dma_start(w2_sb, w2.rearrange("(k p) m -> p k m", p=128))
        wkv_sb = wpool.tile([128, 8, 2 * C], BF16, name="wkv_sb")
        nc.sync.dma_start(wkv_sb, wkv.rearrange("(k p) m -> p k m", p=128))
        if with_bias:
            ones_b = cpool.tile([1, 128], BF16, name="ones_b")
            nc.vector.memset(ones_b, 1.0)
            b2_r = cpool.tile([1, C], BF16, name="b2_r")
            nc.sync.dma_start(b2_r, b2.rearrange("(one f) -> one f", one=1))

        # ctx accumulators: ctxT per head h at cols (h%8)*64, layout [e, d]
        # heads 0-7 in ctx_ps[0], heads 8-15 in ctx_ps[1]; PSUM-resident.
        ctx_ps = [
            ctxp_pool.tile([64, 512], F32, name=f"ctx_ps{i}") for i in range(2)
        ]
        spairs = [spool.tile([128, 128], BF16, name=f"spair{j}") for j in range(8)]

        xqT_tiles = {}

        def emit_trans(xin, out_pool, tag):
            """[128, 4, C] bf16 chunk-majors -> transposed [128, 8, 512]."""
            xT = out_pool.tile([128, 8, R], BF16, name=tag, tag=tag)
            for j in range(8):
                trp = mmps([128, R], BF16)
                for c4 in range(4):
                    nc.tensor.transpose(
                        trp[:, c4 * 128:(c4 + 1) * 128],
                        xin[:, c4, j * 128:(j + 1) * 128],
                        identb_sb,
                    )
                if j % 2 == 0:
                    nc.vector.tensor_copy(xT[:, j, :], trp)
                else:
                    nc.scalar.copy(xT[:, j, :], trp)
            return xT

        def emit_xq_trans(blk):
            xqT_tiles[blk] = emit_trans(xq_tiles.pop(blk), xqT_pool, "xqT")

        # early xq transposes cover the w1/w2 weight-load latency
        emit_xq_trans(0)
        emit_xq_trans(1)

        # =========================================================
        # Phase A: per 512-row block: transpose -> gate1 -> gate2 ->
        #          kv -> ctx accumulation (PSUM)
        # =========================================================
        for blk in range(NBLK):
            # prefetch next xp block / upcoming xq blocks
            if blk + 1 < NBLK:
                xp_tiles[blk + 1] = x_dma(xpin_pool, xp, blk + 1, "xpin")
            if 1 <= blk <= 5:  # xq blocks 2..6 DMA'd one A-block early
                xq_tiles[blk + 1] = x_dma(xqin_pool, xq, blk + 1, "xqin")

            xpT = emit_trans(xp_tiles.pop(blk), xpT_pool, "xpT")

            # ---- gate1: hT[m, n] = relu(b1 + sum_k w1[k,m]^T xpT[k]) ----
            hT = hT_pool.tile([128, 8, R], BF16, name="hT", tag="hT")
            for m in range(8):
                ps = mmps([128, R], F32)
                for k in range(8):
                    nc.tensor.matmul(
                        ps,
                        w1_sb[:, k, m * 128:(m + 1) * 128],
                        xpT[:, k, :],
                        start=(k == 0),
                        stop=(k == 7),
                    )
                nc.scalar.activation(
                    hT[:, m, :], ps, AF.Relu, bias=b1_sb[:, m:m + 1]
                )

            # ---- gate2: g[n, :] = sigmoid(sum_k hT[k]^T w2[k] + b2) ----
            gts = []
            for c4 in range(4):
                gt = g_pool.tile([128, C], BF16, name="gt", tag="gt")
                for t in range(2):
                    ps = mmps([128, 512], F32)
                    for k in range(8):
                        nc.tensor.matmul(
                            ps,
                            hT[:, k, c4 * 128:(c4 + 1) * 128],
                            w2_sb[:, k, t * 512:(t + 1) * 512],
                            start=(k == 0),
                            stop=(k == 7 and not with_bias),
                        )
                    if with_bias:
                        nc.tensor.matmul(
                            ps,
                            ones_b,
                            b2_r[:, t * 512:(t + 1) * 512],
                            start=False,
                            stop=True,
                        )
                    nc.scalar.activation(
                        gt[:, t * 512:(t + 1) * 512], ps, AF.Sigmoid
                    )
                gts.append(gt)

            # ---- kv projection + ctx accumulation, per 128-row chunk ----
            def emit_kv(c4):
                k_bf = k_pool.tile([128, C], BF16, name="k_bf", tag="k_bf")
                vg = vg_pool.tile([128, C], BF16, name="vg", tag="vg")
                for t in range(4):
                    ps = mmps([128, 512], F32)
                    for k in range(8):
                        nc.tensor.matmul(
                            ps,
                            xpT[:, k, c4 * 128:(c4 + 1) * 128],
                            wkv_sb[:, k, t * 512:(t + 1) * 512],
                            start=(k == 0),
                            stop=(k == 7),
                        )
                    if t < 2:
                        nc.scalar.copy(k_bf[:, t * 512:(t + 1) * 512], ps)
                    else:
                        nc.vector.tensor_mul(
                            vg[:, (t - 2) * 512:(t - 1) * 512],
                            ps,
                            gts[c4][:, (t - 2) * 512:(t - 1) * 512],
                        )
                return k_bf, vg

            def emit_ctx(c4, kv_tiles):
                k_bf, vg = kv_tiles
                first = (blk == 0 and c4 == 0)
                last = (blk == NBLK - 1 and c4 == 3)
                for h in range(H):
                    hp, hc = h // 8, h % 8
                    # start=True clears has_written for the WHOLE bank, so only
                    # the very first matmul into each bank may set it.
                    nc.tensor.matmul(
                        ctx_ps[hp][:, hc * 64:(hc + 1) * 64],
                        vg[:, h * D:(h + 1) * D],
                        k_bf[:, h * D:(h + 1) * D],
                        start=(first and hc == 0),
                        stop=last,
                        skip_group_check=True,
                    )

            kvt = {}
            kvt[0] = emit_kv(0)
            kvt[1] = emit_kv(1)
            emit_ctx(0, kvt[0])
            kvt[2] = emit_kv(2)
            emit_ctx(1, kvt[1])
            kvt[3] = emit_kv(3)
            emit_ctx(2, kvt[2])
            emit_ctx(3, kvt[3])

            if 2 <= blk <= 4:  # spread xq transposes through phase A
                emit_xq_trans(blk)

        # covers the softmax serial-chain bubble on PE
        emit_xq_trans(5)

        # =========================================================
        # Softmax over d (free dim of ctxT) + build block-diag S pairs
        # =========================================================
        with ExitStack() as sm:
            smp = sm.enter_context(tc.tile_pool(name="smpool", bufs=1))
            maxs = smp.tile([64, 16], F32, name="maxs")
            sums = smp.tile([64, 16], F32, name="sums")
            for i in range(2):
                nc.vector.tensor_reduce(
                    maxs[:, i * 8:(i + 1) * 8],
                    ctx_ps[i].rearrange("p (b d) -> p b d", b=8),
                    axis=mybir.AxisListType.X,
                    op=mybir.AluOpType.max,
                )
            cmx = smp.tile([64, 1024], F32, name="cmx")
            for i in range(2):
                nc.vector.tensor_sub(
                    cmx.rearrange("p (h d) -> p h d", h=16)[:, i * 8:(i + 1) * 8, :],
                    ctx_ps[i].rearrange("p (h d) -> p h d", h=8),
                    maxs[:, i * 8:(i + 1) * 8].unsqueeze(-1).broadcast_to(
                        [64, 8, 64]
                    ),
                )
            ets = [mmps([64, 512], F32) for _ in range(2)]
            for i in range(2):
                nc.scalar.activation(
                    ets[i], cmx[:, i * 512:(i + 1) * 512], AF.Exp,
                    scale=float(SCALE),
                )
                nc.vector.tensor_reduce(
                    sums[:, i * 8:(i + 1) * 8],
                    ets[i].rearrange("p (b d) -> p b d", b=8),
                    axis=mybir.AxisListType.X,
                    op=mybir.AluOpType.add,
                )
            recs = smp.tile([64, 16], F32, name="recs")
            nc.vector.reciprocal(recs, sums)
            st = cmx  # reuse
            for i in range(2):
                nc.vector.tensor_mul(
                    st.rearrange("p (h d) -> p h d", h=16)[:, i * 8:(i + 1) * 8, :],
                    ets[i].rearrange("p (h d) -> p h d", h=8),
                    recs[:, i * 8:(i + 1) * 8].unsqueeze(-1).broadcast_to(
                        [64, 8, 64]
                    ),
                )
            # st: softmaxed ctxT [e, d] per head at cols h*64.  Transposing the
            # side-by-side pair [ctxT_2j | ctxT_2j+1] ([64, 128]) gives
            # [S_2j stacked above S_2j+1] ([128, 64]); scatter to block-diag.
            zero_sb = smp.tile([128, 128], BF16, name="zero_sb")
            nc.vector.memset(zero_sb, 0.0)
            for j in range(8):
                tp = mmps([128, 64], F32)
                nc.tensor.transpose(
                    tp, st[:, (2 * j) * 64:(2 * j + 2) * 64], ident64_sb
                )
                nc.vector.tensor_copy(spairs[j], zero_sb)
                nc.vector.tensor_copy(spairs[j][0:64, 0:64], tp[0:64, :])
                nc.vector.tensor_copy(spairs[j][64:128, 64:128], tp[64:128, :])

        # =========================================================
        # Phase B: o[nchunk, j*128:(j+1)*128] = (xqT_j_chunk).T @ spair_j
        # =========================================================
        def emit_b(blk):
            xqT = xqT_tiles.pop(blk)
            oout = oout_pool.tile([128, 4, C], BF16, name="oo", tag="oo")
            for c4 in range(4):
                for half in range(2):
                    ps = mmps([128, 512], F32)
                    for jj in range(4):
                        j = half * 4 + jj
                        nc.tensor.matmul(
                            ps[:, jj * 128:(jj + 1) * 128],
                            xqT[:, j, c4 * 128:(c4 + 1) * 128],
                            spairs[j],
                            start=True,
                            stop=True,
                            skip_group_check=True,
                        )
                    if half == 0:
                        nc.vector.tensor_copy(oout[:, c4, 0:512], ps)
                    else:
                        nc.scalar.copy(oout[:, c4, 512:1024], ps)
            nc.sync.dma_start(
                o[blk * R:(blk + 1) * R, :].rearrange("(c p) m -> p c m", p=128),
                oout,
            )

        emit_b(0)
        emit_xq_trans(6)
        xq_tiles[7] = x_dma(xqin_pool, xq, 7, "xqin")
        emit_b(1)
        emit_xq_trans(7)
        for blk in range(2, NBLK):
            emit_b(blk)

    nc.compile()
    return nc


def _get_program(with_bias=False):
    key = ("nc", bool(with_bias))
    if key not in _CACHE:
        _CACHE[key] = _build_program(with_bias)
    return _CACHE[key]


def make_in_maps(x1, x2, Wkv1, Wkv2, g1_w1, g1_b1, g1_w2, g1_b2,
                 g2_w1, g2_b1, g2_w2, g2_b2):
    """Core (s, b): cores 0-3 = (s=0, b), cores 4-7 = (s=1, b)."""
    import ml_dtypes
    BF = ml_dtypes.bfloat16
    identb = np.eye(128, dtype=BF)
    ident64 = np.eye(64, dtype=np.float32)

    def bf(a):
        return np.ascontiguousarray(np.asarray(a, np.float32).astype(BF))

    x1b = [bf(x1[b]) for b in range(x1.shape[0])]
    x2b = [bf(x2[b]) for b in range(x2.shape[0])]
    Ws = [
        dict(wkv=bf(Wkv1), w1=bf(g1_w1), b1=np.asarray(g1_b1, np.float32),
             w2=bf(g1_w2), b2=bf(g1_b2)),
        dict(wkv=bf(Wkv2), w1=bf(g2_w1), b1=np.asarray(g2_b1, np.float32),
             w2=bf(g2_w2), b2=bf(g2_b2)),
    ]
    in_maps = []
    for core in range(8):
        s, b = core // 4, core % 4
        m = dict(Ws[s])
        m["xp"] = x1b[b] if s == 0 else x2b[b]
        m["xq"] = x2b[b] if s == 0 else x1b[b]
        m["identb"] = identb
        m["ident64"] = ident64
        in_maps.append(m)
    return in_maps


def kernel(x1, x2, Wkv1, Wkv2, g1_w1, g1_b1, g1_w2, g1_b2,
           g2_w1, g2_b1, g2_w2, g2_b2, _runner=None):
    """Full-input entry point.  Returns (o1, o2), each [4, 4096, 1024] f32."""
    from concourse.bass_utils import run_bass_kernel_spmd

    args = [np.asarray(a, dtype=np.float32) for a in
            (x1, x2, Wkv1, Wkv2, g1_w1, g1_b1, g1_w2, g1_b2,
             g2_w1, g2_b1, g2_w2, g2_b2)]
    with_bias = bool(np.any(args[7]) or np.any(args[11]))  # g1_b2, g2_b2
    nc = _get_program(with_bias)
    in_maps = make_in_maps(*args)
    if _runner is None:
        res = run_bass_kernel_spmd(nc, in_maps, core_ids=list(range(8)))
        results = res.results
    else:
        results = _runner(nc, in_maps)

    B = x1.shape[0]
    o1 = np.empty((B, N, C), dtype=np.float32)
    o2 = np.empty((B, N, C), dtype=np.float32)
    for core in range(8):
        s, b = core // 4, core % 4
        out = np.asarray(results[core]["o"], dtype=np.float32)
        if s == 0:
            o2[b] = out   # core projected x1 -> ctx1 -> o2 = q2 @ ctx1
        else:
            o1[b] = out
    return (o1, o2)


# revision 11
# speedup vs baseline: 1.3536x; 1.0419x over previous
"""Trainium2 Bass kernel for nn_CrossAttention (dense_transformer).

Reference computation (per batch b, per stream s in {1,2}):
    q_s   = heads(x_s)                      # [H, N, D] slices of x_s
    kv_s  = x_s @ Wkv_s -> k_s, v_s         # [N, C] each
    gate_s= sigmoid(relu(x_s @ w1 + b1) @ w2 + b2)
    ctx_s = softmax_d( scale * k_s^T @ (v_s * gate_s) )   # [H, D, D], softmax over d
    o_1   = q_1 @ ctx_2 ; o_2 = q_2 @ ctx_1  (cross)

Sharding: 8 cores = (stream s, batch b) pairs.  Core (s, b) projects
x_s[b] (kv + gate + ctx_s[b]) and then computes the OTHER stream's
output o_{1-s}[b] = q_{1-s}[b] @ softmax(ctx_s[b]).  No cross-core
communication; host concatenates outputs.

Fully-fused single streaming pass (no DRAM spills).  All matmul
operands bf16 (host-converted); PSUM-resident ctx accumulation with
(h, h+8) head pairs packed into distinct PE column groups; output
written bf16 and upcast on host.
"""

import numpy as np
from contextlib import ExitStack

N = 4096
C = 1024
H = 16
D = 64
SCALE = D ** (-0.5)
R = 512              # rows per A-phase block
NBLK = N // R        # 8 blocks
KCH = C // 128       # 8 contraction chunks

_CACHE = {}


def _build_program(with_bias):
    """Build the SPMD Bass program (same for all 8 cores)."""
    import concourse.bass as bass
    import concourse.bacc as bacc
    import concourse.tile as tile
    import concourse.mybir as mybir

    F32 = mybir.dt.float32
    BF16 = mybir.dt.bfloat16
    AF = mybir.ActivationFunctionType

    nc = bacc.Bacc("TRN2", target_bir_lowering=False, debug=False, num_devices=8)

    xp = nc.dram_tensor("xp", [N, C], BF16, kind="ExternalInput").ap()
    xq = nc.dram_tensor("xq", [N, C], BF16, kind="ExternalInput").ap()
    wkv = nc.dram_tensor("wkv", [C, 2 * C], BF16, kind="ExternalInput").ap()
    w1 = nc.dram_tensor("w1", [C, C], BF16, kind="ExternalInput").ap()
    b1 = nc.dram_tensor("b1", [C], F32, kind="ExternalInput").ap()
    w2 = nc.dram_tensor("w2", [C, C], BF16, kind="ExternalInput").ap()
    b2 = nc.dram_tensor("b2", [C], BF16, kind="ExternalInput").ap()
    identb = nc.dram_tensor("identb", [128, 128], BF16, kind="ExternalInput").ap()
    # identity replicated on both partition halves: ident2[p, c] = (p % 64 == c)
    ident2 = nc.dram_tensor("ident2", [128, 64], F32, kind="ExternalInput").ap()
    o = nc.dram_tensor("o", [N, C], BF16, kind="ExternalOutput").ap()

    with tile.TileContext(nc) as tc, ExitStack() as ctx:
        # ---------- pools ----------
        cpool = ctx.enter_context(tc.tile_pool(name="consts", bufs=1))
        wpool = ctx.enter_context(tc.tile_pool(name="weights", bufs=1))
        ctxp_pool = ctx.enter_context(tc.tile_pool(name="ctxps", bufs=1, space="PSUM"))
        mmps_pool = ctx.enter_context(tc.tile_pool(name="mmps", bufs=6, space="PSUM"))
        spool = ctx.enter_context(tc.tile_pool(name="spairs", bufs=1))
        xpin_pool = ctx.enter_context(tc.tile_pool(name="xpin", bufs=2))
        xqin_pool = ctx.enter_context(tc.tile_pool(name="xqin", bufs=2))
        xpT_pool = ctx.enter_context(tc.tile_pool(name="xpT", bufs=1))
        hT_pool = ctx.enter_context(tc.tile_pool(name="hT", bufs=1))
        g_pool = ctx.enter_context(tc.tile_pool(name="g", bufs=4))
        k_pool = ctx.enter_context(tc.tile_pool(name="k", bufs=2))
        vg_pool = ctx.enter_context(tc.tile_pool(name="vg", bufs=2))
        xqT_pool = ctx.enter_context(tc.tile_pool(name="xqT", bufs=6))
        oout_pool = ctx.enter_context(tc.tile_pool(name="oout", bufs=4))

        def mmps(shape, dtype):
            return mmps_pool.tile(shape, dtype, name="mmps", tag="mmps")

        # ---------- DMA priority order ----------
        identb_sb = cpool.tile([128, 128], BF16, name="identb_sb")
        nc.sync.dma_start(identb_sb, identb)
        ident2_sb = cpool.tile([128, 64], F32, name="ident2_sb")
        nc.sync.dma_start(ident2_sb, ident2)
        b1_sb = cpool.tile([128, 8], F32, name="b1_sb")  # b1_sb[p, m] = b1[m*128+p]
        nc.sync.dma_start(b1_sb, b1.rearrange("(m p) -> p m", p=128))

        def x_dma(pool, src, blk, tag):
            t = pool.tile([128, 4, C], BF16, name=tag, tag=tag)
            nc.sync.dma_start(
                t, src[blk * R:(blk + 1) * R, :].rearrange("(c p) m -> p c m", p=128)
            )
            return t

        # xp block 0 and w1 are on the critical path: first
        xp_tiles = {0: x_dma(xpin_pool, xp, 0, "xpin")}
        w1_sb = wpool.tile([128, 8, C], BF16, name="w1_sb")  # [p, k, m]
        nc.sync.dma_start(w1_sb, w1.rearrange("(k p) m -> p k m", p=128))
        xq_tiles = {0: x_dma(xqin_pool, xq, 0, "xqin"),
                    1: x_dma(xqin_pool, xq, 1, "xqin")}
        w2_sb = wpool.tile([128, 8, C], BF16, name="w2_sb")
        nc.sync.dma_start(w2_sb, w2.rearrange("(k p) m -> p k m", p=128))
        wkv_sb = wpool.tile([128, 8, 2 * C], BF16, name="wkv_sb")
        nc.sync.dma_start(wkv_sb, wkv.rearrange("(k p) m -> p k m", p=128))
        if with_bias:
            ones_b = cpool.tile([1, 128], BF16, name="ones_b")
            nc.vector.memset(ones_b, 1.0)
            b2_r = cpool.tile([1, C], BF16, name="b2_r")
            nc.sync.dma_start(b2_r, b2.rearrange("(one f) -> one f", one=1))

        # ctx accumulators, ctxT layout [e, d] per head at cols (h%8)*64.
        # Bank A: heads 0-7 on partitions 0-63 (PE col groups 0-1).
        # Bank B: heads 8-15 on partitions 64-127 (PE col groups 2-3).
        # The (h, h+8) matmul pairs run concurrently on the PE.
        ctx_psA = ctxp_pool.tile([128, 512], F32, name="ctx_psA")
        ctx_psB = ctxp_pool.tile([128, 512], F32, name="ctx_psB")
        spairs = [spool.tile([128, 128], BF16, name=f"spair{j}") for j in range(8)]

        xqT_tiles = {}

        def emit_trans(xin, out_pool, tag):
            """[128, 4, C] bf16 chunk-major -> transposed [128, 8, 512]."""
            xT = out_pool.tile([128, 8, R], BF16, name=tag, tag=tag)
            for j in range(8):
                trp = mmps([128, R], BF16)
                for c4 in range(4):
                    nc.tensor.transpose(
                        trp[:, c4 * 128:(c4 + 1) * 128],
                        xin[:, c4, j * 128:(j + 1) * 128],
                        identb_sb,
                    )
                if j % 2 == 0:
                    nc.vector.tensor_copy(xT[:, j, :], trp)
                else:
                    nc.scalar.copy(xT[:, j, :], trp)
            return xT

        def emit_xq_trans(blk):
            xqT_tiles[blk] = emit_trans(xq_tiles.pop(blk), xqT_pool, "xqT")

        # =========================================================
        # Phase A: per 512-row block: transpose -> gate1 -> gate2 ->
        #          kv -> ctx accumulation (PSUM)
        # =========================================================
        for blk in range(NBLK):
            # prefetch next xp block / upcoming xq blocks
            if blk + 1 < NBLK:
                xp_tiles[blk + 1] = x_dma(xpin_pool, xp, blk + 1, "xpin")
            if 1 <= blk <= 5:  # xq blocks 2..6 DMA'd one A-block early
                xq_tiles[blk + 1] = x_dma(xqin_pool, xq, blk + 1, "xqin")

            xpT = emit_trans(xp_tiles.pop(blk), xpT_pool, "xpT")

            # ---- gate1: hT[m, n] = relu(b1 + sum_k w1[k,m]^T xpT[k]) ----
            hT = hT_pool.tile([128, 8, R], BF16, name="hT", tag="hT")
            for m in range(8):
                ps = mmps([128, R], F32)
                for k in range(8):
                    nc.tensor.matmul(
                        ps,
                        w1_sb[:, k, m * 128:(m + 1) * 128],
                        xpT[:, k, :],
                        start=(k == 0),
                        stop=(k == 7),
                    )
                nc.scalar.activation(
                    hT[:, m, :], ps, AF.Relu, bias=b1_sb[:, m:m + 1]
                )

            if blk == 0:
                # xq transposes here cover the w2/wkv weight-load latency
                emit_xq_trans(0)
                emit_xq_trans(1)

            # ---- gate2: g[n, :] = sigmoid(sum_k hT[k]^T w2[k] + b2) ----
            gts = []
            for c4 in range(4):
                gt = g_pool.tile([128, C], BF16, name="gt", tag="gt")
                for t in range(2):
                    ps = mmps([128, 512], F32)
                    for k in range(8):
                        nc.tensor.matmul(
                            ps,
                            hT[:, k, c4 * 128:(c4 + 1) * 128],
                            w2_sb[:, k, t * 512:(t + 1) * 512],
                            start=(k == 0),
                            stop=(k == 7 and not with_bias),
                        )
                    if with_bias:
                        nc.tensor.matmul(
                            ps,
                            ones_b,
                            b2_r[:, t * 512:(t + 1) * 512],
                            start=False,
                            stop=True,
                        )
                    nc.scalar.activation(
                        gt[:, t * 512:(t + 1) * 512], ps, AF.Sigmoid
                    )
                gts.append(gt)

            # ---- kv projection + ctx accumulation, per 128-row chunk ----
            def emit_kv(c4):
                k_bf = k_pool.tile([128, C], BF16, name="k_bf", tag="k_bf")
                vg = vg_pool.tile([128, C], BF16, name="vg", tag="vg")
                for t in range(4):
                    ps = mmps([128, 512], F32)
                    for k in range(8):
                        nc.tensor.matmul(
                            ps,
                            xpT[:, k, c4 * 128:(c4 + 1) * 128],
                            wkv_sb[:, k, t * 512:(t + 1) * 512],
                            start=(k == 0),
                            stop=(k == 7),
                        )
                    if t < 2:
                        nc.scalar.copy(k_bf[:, t * 512:(t + 1) * 512], ps)
                    else:
                        nc.vector.tensor_mul(
                            vg[:, (t - 2) * 512:(t - 1) * 512],
                            ps,
                            gts[c4][:, (t - 2) * 512:(t - 1) * 512],
                        )
                return k_bf, vg

            def emit_ctx(c4, kv_tiles):
                k_bf, vg = kv_tiles
                first = (blk == 0 and c4 == 0)
                last = (blk == NBLK - 1 and c4 == 3)
                for hc in range(8):
                    for hp in range(2):
                        h = hp * 8 + hc
                        dst = ctx_psA if hp == 0 else ctx_psB
                        # start=True clears has_written for the whole bank ->
                        # exactly one clearing matmul per bank.
                        nc.tensor.matmul(
                            dst[hp * 64:(hp + 1) * 64, hc * 64:(hc + 1) * 64],
                            vg[:, h * D:(h + 1) * D],
                            k_bf[:, h * D:(h + 1) * D],
                            start=(first and hc == 0),
                            stop=last,
                            skip_group_check=True,
                            tile_position=(0, hp * 64),
                        )

            kvt = {}
            kvt[0] = emit_kv(0)
            kvt[1] = emit_kv(1)
            emit_ctx(0, kvt[0])
            kvt[2] = emit_kv(2)
            emit_ctx(1, kvt[1])
            kvt[3] = emit_kv(3)
            emit_ctx(2, kvt[2])
            emit_ctx(3, kvt[3])

            if 2 <= blk <= 4:  # spread xq transposes through phase A
                emit_xq_trans(blk)

        # covers part of the softmax serial-chain bubble on PE
        emit_xq_trans(5)
        xq_tiles[7] = x_dma(xqin_pool, xq, 7, "xqin")

        # =========================================================
        # Softmax over d (free dim of ctxT) + build block-diag S pairs
        # st layout: heads 0-7 on partitions 0-63, heads 8-15 on 64-127.
        # =========================================================
        with ExitStack() as sm:
            smp = sm.enter_context(tc.tile_pool(name="smpool", bufs=1))
            maxs = smp.tile([128, 8], F32, name="maxs")
            sums = smp.tile([128, 8], F32, name="sums")
            cmx = smp.tile([128, 512], F32, name="cmx")
            halves = [(ctx_psA, slice(0, 64)), (ctx_psB, slice(64, 128))]
            for cps, sl in halves:
                nc.vector.tensor_reduce(
                    maxs[sl, :],
                    cps[sl, :].rearrange("p (b d) -> p b d", b=8),
                    axis=mybir.AxisListType.X,
                    op=mybir.AluOpType.max,
                )
                nc.vector.tensor_sub(
                    cmx[sl, :].rearrange("p (h d) -> p h d", h=8),
                    cps[sl, :].rearrange("p (h d) -> p h d", h=8),
                    maxs[sl, :].unsqueeze(-1).broadcast_to([64, 8, 64]),
                )
            et = mmps([128, 512], F32)
            nc.scalar.activation(et, cmx, AF.Exp, scale=float(SCALE))
            nc.vector.tensor_reduce(
                sums,
                et.rearrange("p (b d) -> p b d", b=8),
                axis=mybir.AxisListType.X,
                op=mybir.AluOpType.add,
            )
            recs = smp.tile([128, 8], F32, name="recs")
            nc.vector.reciprocal(recs, sums)
            st = cmx  # reuse
            nc.vector.tensor_mul(
                st.rearrange("p (h d) -> p h d", h=8),
                et.rearrange("p (h d) -> p h d", h=8),
                recs.unsqueeze(-1).broadcast_to([128, 8, 64]),
            )
            # Transposing the pair [ctxT_2j | ctxT_2j+1] ([64, 128]) gives
            # [S_2j stacked above S_2j+1] ([128, 64]); scatter to block-diag.
            zero_sb = smp.tile([128, 128], BF16, name="zero_sb")
            nc.vector.memset(zero_sb, 0.0)
            for j in range(8):
                sl = slice(0, 64) if j < 4 else slice(64, 128)
                col = (2 * j) * 64 % 512
                tp = mmps([128, 64], F32)
                nc.tensor.transpose(
                    tp, st[sl, col:col + 128], ident2_sb[sl, :]
                )
                nc.vector.tensor_copy(spairs[j], zero_sb)
                nc.vector.tensor_copy(spairs[j][0:64, 0:64], tp[0:64, :])
                nc.vector.tensor_copy(spairs[j][64:128, 64:128], tp[64:128, :])

        # =========================================================
        # Phase B: o[nchunk, j*128:(j+1)*128] = (xqT_j_chunk).T @ spair_j
        # =========================================================
        def emit_b(blk):
            xqT = xqT_tiles.pop(blk)
            for ch in range(2):  # half-block output granularity
                oout = oout_pool.tile([128, 2, C], BF16, name="oo", tag="oo")
                for cc in range(2):
                    c4 = ch * 2 + cc
                    for half in range(2):
                        ps = mmps([128, 512], F32)
                        for jj in range(4):
                            j = half * 4 + jj
                            nc.tensor.matmul(
                                ps[:, jj * 128:(jj + 1) * 128],
                                xqT[:, j, c4 * 128:(c4 + 1) * 128],
                                spairs[j],
                                start=True,
                                stop=True,
                                skip_group_check=True,
                            )
                        if half == 0:
                            nc.vector.tensor_copy(oout[:, cc, 0:512], ps)
                        else:
                            nc.scalar.copy(oout[:, cc, 512:1024], ps)
                nc.sync.dma_start(
                    o[blk * R + ch * 256:blk * R + (ch + 1) * 256, :].rearrange(
                        "(c p) m -> p c m", p=128
                    ),
                    oout,
                )

        emit_b(0)
        emit_xq_trans(6)
        emit_b(1)
        emit_xq_trans(7)
        for blk in range(2, NBLK):
            emit_b(blk)

    nc.compile()
    return nc


def _get_program(with_bias=False):
    key = ("nc", bool(with_bias))
    if key not in _CACHE:
        _CACHE[key] = _build_program(with_bias)
    return _CACHE[key]


def make_in_maps(x1, x2, Wkv1, Wkv2, g1_w1, g1_b1, g1_w2, g1_b2,
                 g2_w1, g2_b1, g2_w2, g2_b2):
    """Core (s, b): cores 0-3 = (s=0, b), cores 4-7 = (s=1, b)."""
    import ml_dtypes
    BF = ml_dtypes.bfloat16
    identb = np.eye(128, dtype=BF)
    eye64 = np.eye(64, dtype=np.float32)
    ident2 = np.ascontiguousarray(np.concatenate([eye64, eye64], axis=0))

    def bf(a):
        return np.ascontiguousarray(np.asarray(a, np.float32).astype(BF))

    x1b = [bf(x1[b]) for b in range(x1.shape[0])]
    x2b = [bf(x2[b]) for b in range(x2.shape[0])]
    Ws = [
        dict(wkv=bf(Wkv1), w1=bf(g1_w1), b1=np.asarray(g1_b1, np.float32),
             w2=bf(g1_w2), b2=bf(g1_b2)),
        dict(wkv=bf(Wkv2), w1=bf(g2_w1), b1=np.asarray(g2_b1, np.float32),
             w2=bf(g2_w2), b2=bf(g2_b2)),
    ]
    in_maps = []
    for core in range(8):
        s, b = core // 4, core % 4
        m = dict(Ws[s])
        m["xp"] = x1b[b] if s == 0 else x2b[b]
        m["xq"] = x2b[b] if s == 0 else x1b[b]
        m["identb"] = identb
        m["ident2"] = ident2
        in_maps.append(m)
    return in_maps


def kernel(x1, x2, Wkv1, Wkv2, g1_w1, g1_b1, g1_w2, g1_b2,
           g2_w1, g2_b1, g2_w2, g2_b2, _runner=None):
    """Full-input entry point.  Returns (o1, o2), each [4, 4096, 1024] f32."""
    from concourse.bass_utils import run_bass_kernel_spmd

    args = [np.asarray(a, dtype=np.float32) for a in
            (x1, x2, Wkv1, Wkv2, g1_w1, g1_b1, g1_w2, g1_b2,
             g2_w1, g2_b1, g2_w2, g2_b2)]
    with_bias = bool(np.any(args[7]) or np.any(args[11]))  # g1_b2, g2_b2
    nc = _get_program(with_bias)
    in_maps = make_in_maps(*args)
    if _runner is None:
        res = run_bass_kernel_spmd(nc, in_maps, core_ids=list(range(8)))
        results = res.results
    else:
        results = _runner(nc, in_maps)

    B = x1.shape[0]
    o1 = np.empty((B, N, C), dtype=np.float32)
    o2 = np.empty((B, N, C), dtype=np.float32)
    for core in range(8):
        s, b = core // 4, core % 4
        out = np.asarray(results[core]["o"], dtype=np.float32)
        if s == 0:
            o2[b] = out   # core projected x1 -> ctx1 -> o2 = q2 @ ctx1
        else:
            o1[b] = out
    return (o1, o2)


# revision 16
# speedup vs baseline: 1.3605x; 1.0051x over previous
"""Trainium2 Bass kernel for nn_CrossAttention (dense_transformer).

Reference computation (per batch b, per stream s in {1,2}):
    q_s   = heads(x_s)                      # [H, N, D] slices of x_s
    kv_s  = x_s @ Wkv_s -> k_s, v_s         # [N, C] each
    gate_s= sigmoid(relu(x_s @ w1 + b1) @ w2 + b2)
    ctx_s = softmax_d( scale * k_s^T @ (v_s * gate_s) )   # [H, D, D], softmax over d
    o_1   = q_1 @ ctx_2 ; o_2 = q_2 @ ctx_1  (cross)

Sharding: 8 cores = (stream s, batch b) pairs.  Core (s, b) projects
x_s[b] (kv + gate + ctx_s[b]) and then computes the OTHER stream's
output o_{1-s}[b] = q_{1-s}[b] @ softmax(ctx_s[b]).  No cross-core
communication; host concatenates outputs.

Fully-fused single streaming pass (no DRAM spills).  All matmul
operands bf16 (host-converted); PSUM-resident ctx accumulation with
(h, h+8) head pairs packed into distinct PE column groups; output
written bf16 and upcast on host.
"""

import numpy as np
from contextlib import ExitStack

N = 4096
C = 1024
H = 16
D = 64
SCALE = D ** (-0.5)
R = 512              # rows per A-phase block
NBLK = N // R        # 8 blocks
KCH = C // 128       # 8 contraction chunks

_CACHE = {}


def _build_program(with_bias):
    """Build the SPMD Bass program (same for all 8 cores)."""
    import concourse.bass as bass
    import concourse.bacc as bacc
    import concourse.tile as tile
    import concourse.mybir as mybir

    F32 = mybir.dt.float32
    BF16 = mybir.dt.bfloat16
    AF = mybir.ActivationFunctionType

    nc = bacc.Bacc("TRN2", target_bir_lowering=False, debug=False, num_devices=8)

    xp = nc.dram_tensor("xp", [N, C], BF16, kind="ExternalInput").ap()
    xq = nc.dram_tensor("xq", [N, C], BF16, kind="ExternalInput").ap()
    wkv = nc.dram_tensor("wkv", [C, 2 * C], BF16, kind="ExternalInput").ap()
    w1 = nc.dram_tensor("w1", [C, C], BF16, kind="ExternalInput").ap()
    b1 = nc.dram_tensor("b1", [C], F32, kind="ExternalInput").ap()
    w2 = nc.dram_tensor("w2", [C, C], BF16, kind="ExternalInput").ap()
    b2 = nc.dram_tensor("b2", [C], BF16, kind="ExternalInput").ap()
    identb = nc.dram_tensor("identb", [128, 128], BF16, kind="ExternalInput").ap()
    # identity replicated on both partition halves: ident2[p, c] = (p % 64 == c)
    ident2 = nc.dram_tensor("ident2", [128, 64], F32, kind="ExternalInput").ap()
    o = nc.dram_tensor("o", [N, C], BF16, kind="ExternalOutput").ap()

    with tile.TileContext(nc) as tc, ExitStack() as ctx:
        # ---------- pools ----------
        cpool = ctx.enter_context(tc.tile_pool(name="consts", bufs=1))
        wpool = ctx.enter_context(tc.tile_pool(name="weights", bufs=1))
        ctxp_pool = ctx.enter_context(tc.tile_pool(name="ctxps", bufs=1, space="PSUM"))
        mmps_pool = ctx.enter_context(tc.tile_pool(name="mmps", bufs=6, space="PSUM"))
        spool = ctx.enter_context(tc.tile_pool(name="spairs", bufs=1))
        xpin_pool = ctx.enter_context(tc.tile_pool(name="xpin", bufs=2))
        xqin_pool = ctx.enter_context(tc.tile_pool(name="xqin", bufs=2))
        xpT_pool = ctx.enter_context(tc.tile_pool(name="xpT", bufs=1))
        hT_pool = ctx.enter_context(tc.tile_pool(name="hT", bufs=1))
        g_pool = ctx.enter_context(tc.tile_pool(name="g", bufs=4))
        k_pool = ctx.enter_context(tc.tile_pool(name="k", bufs=2))
        vg_pool = ctx.enter_context(tc.tile_pool(name="vg", bufs=2))
        xqT_pool = ctx.enter_context(tc.tile_pool(name="xqT", bufs=6))
        oout_pool = ctx.enter_context(tc.tile_pool(name="oout", bufs=4))

        def mmps(shape, dtype):
            return mmps_pool.tile(shape, dtype, name="mmps", tag="mmps")

        # ---------- DMA priority order ----------
        identb_sb = cpool.tile([128, 128], BF16, name="identb_sb")
        nc.sync.dma_start(identb_sb, identb)
        ident2_sb = cpool.tile([128, 64], F32, name="ident2_sb")
        nc.sync.dma_start(ident2_sb, ident2)
        b1_sb = cpool.tile([128, 8], F32, name="b1_sb")  # b1_sb[p, m] = b1[m*128+p]
        nc.sync.dma_start(b1_sb, b1.rearrange("(m p) -> p m", p=128))

        def x_dma(pool, src, blk, tag):
            t = pool.tile([128, 4, C], BF16, name=tag, tag=tag)
            nc.sync.dma_start(
                t, src[blk * R:(blk + 1) * R, :].rearrange("(c p) m -> p c m", p=128)
            )
            return t

        # xp block 0 and w1 are on the critical path: first, finely split
        def x_dma_half(pool, src, blk, hf, tag):
            t = pool.tile([128, 2, C], BF16, name=tag, tag=tag)
            nc.sync.dma_start(
                t,
                src[blk * R + hf * 256:blk * R + (hf + 1) * 256, :].rearrange(
                    "(c p) m -> p c m", p=128
                ),
            )
            return t

        xp0_halves = [x_dma_half(xpin_pool, xp, 0, hf, "xpin") for hf in range(2)]
        xp_tiles = {}
        w1_sb = wpool.tile([128, 8, C], BF16, name="w1_sb")  # [p, k, m]
        w1r = w1.rearrange("(k p) m -> p k m", p=128)
        for mh in range(2):
            nc.sync.dma_start(
                w1_sb[:, :, mh * 512:(mh + 1) * 512], w1r[:, :, mh * 512:(mh + 1) * 512]
            )
        xq_tiles = {0: x_dma(xqin_pool, xq, 0, "xqin"),
                    1: x_dma(xqin_pool, xq, 1, "xqin")}
        w2_sb = wpool.tile([128, 8, C], BF16, name="w2_sb")
        nc.sync.dma_start(w2_sb, w2.rearrange("(k p) m -> p k m", p=128))
        wkv_sb = wpool.tile([128, 8, 2 * C], BF16, name="wkv_sb")
        nc.sync.dma_start(wkv_sb, wkv.rearrange("(k p) m -> p k m", p=128))
        if with_bias:
            ones_b = cpool.tile([1, 128], BF16, name="ones_b")
            nc.vector.memset(ones_b, 1.0)
            b2_r = cpool.tile([1, C], BF16, name="b2_r")
            nc.sync.dma_start(b2_r, b2.rearrange("(one f) -> one f", one=1))

        # ctx accumulators, ctxT layout [e, d] per head at cols (h%8)*64.
        # Bank A: heads 0-7 on partitions 0-63 (PE col groups 0-1).
        # Bank B: heads 8-15 on partitions 64-127 (PE col groups 2-3).
        # The (h, h+8) matmul pairs run concurrently on the PE.
        ctx_psA = ctxp_pool.tile([128, 512], F32, name="ctx_psA")
        ctx_psB = ctxp_pool.tile([128, 512], F32, name="ctx_psB")
        spairs = [spool.tile([128, 128], BF16, name=f"spair{j}") for j in range(8)]
        for j in range(8):  # pre-zero; softmax writes only the diagonal blocks
            nc.vector.memset(spairs[j], 0.0)

        xqT_tiles = {}

        def emit_trans_fn(chunk, out_pool, tag):
            """chunk(c4) -> [128, C] bf16 slice; -> transposed [128, 8, 512]."""
            xT = out_pool.tile([128, 8, R], BF16, name=tag, tag=tag)
            for j in range(8):
                trp = mmps([128, R], BF16)
                for c4 in range(4):
                    nc.tensor.transpose(
                        trp[:, c4 * 128:(c4 + 1) * 128],
                        chunk(c4)[:, j * 128:(j + 1) * 128],
                        identb_sb,
                    )
                if j % 2 == 0:
                    nc.vector.tensor_copy(xT[:, j, :], trp)
                else:
                    nc.scalar.copy(xT[:, j, :], trp)
            return xT

        def emit_trans(xin, out_pool, tag):
            return emit_trans_fn(lambda c4: xin[:, c4, :], out_pool, tag)

        def emit_xq_trans(blk):
            xqT_tiles[blk] = emit_trans(xq_tiles.pop(blk), xqT_pool, "xqT")

        # =========================================================
        # Phase A: per 512-row block: transpose -> gate1 -> gate2 ->
        #          kv -> ctx accumulation (PSUM)
        # =========================================================
        for blk in range(NBLK):
            # prefetch next xp block / upcoming xq blocks
            if blk + 1 < NBLK:
                xp_tiles[blk + 1] = x_dma(xpin_pool, xp, blk + 1, "xpin")
            if 1 <= blk <= 5:  # xq blocks 2..6 DMA'd one A-block early
                xq_tiles[blk + 1] = x_dma(xqin_pool, xq, blk + 1, "xqin")

            if blk == 0:
                xpT = emit_trans_fn(
                    lambda c4: xp0_halves[c4 // 2][:, c4 % 2, :], xpT_pool, "xpT"
                )
            else:
                xpT = emit_trans(xp_tiles.pop(blk), xpT_pool, "xpT")

            # ---- gate1: hT[m, n] = relu(b1 + sum_k w1[k,m]^T xpT[k]) ----
            hT = hT_pool.tile([128, 8, R], BF16, name="hT", tag="hT")
            for m in range(8):
                ps = mmps([128, R], F32)
                for k in range(8):
                    nc.tensor.matmul(
                        ps,
                        w1_sb[:, k, m * 128:(m + 1) * 128],
                        xpT[:, k, :],
                        start=(k == 0),
                        stop=(k == 7),
                    )
                nc.scalar.activation(
                    hT[:, m, :], ps, AF.Relu, bias=b1_sb[:, m:m + 1]
                )

            if blk == 0:
                # xq transposes here cover the w2/wkv weight-load latency
                emit_xq_trans(0)
                emit_xq_trans(1)

            # ---- gate2: g[n, :] = sigmoid(sum_k hT[k]^T w2[k] + b2) ----
            gts = []
            for c4 in range(4):
                gt = g_pool.tile([128, C], BF16, name="gt", tag="gt")
                for t in range(2):
                    ps = mmps([128, 512], F32)
                    for k in range(8):
                        nc.tensor.matmul(
                            ps,
                            hT[:, k, c4 * 128:(c4 + 1) * 128],
                            w2_sb[:, k, t * 512:(t + 1) * 512],
                            start=(k == 0),
                            stop=(k == 7 and not with_bias),
                        )
                    if with_bias:
                        nc.tensor.matmul(
                            ps,
                            ones_b,
                            b2_r[:, t * 512:(t + 1) * 512],
                            start=False,
                            stop=True,
                        )
                    nc.scalar.activation(
                        gt[:, t * 512:(t + 1) * 512], ps, AF.Sigmoid
                    )
                gts.append(gt)

            # ---- kv projection + ctx accumulation, per 128-row chunk ----
            def emit_kv(c4):
                k_bf = k_pool.tile([128, C], BF16, name="k_bf", tag="k_bf")
                vg = vg_pool.tile([128, C], BF16, name="vg", tag="vg")
                for t in range(4):
                    ps = mmps([128, 512], F32)
                    for k in range(8):
                        nc.tensor.matmul(
                            ps,
                            xpT[:, k, c4 * 128:(c4 + 1) * 128],
                            wkv_sb[:, k, t * 512:(t + 1) * 512],
                            start=(k == 0),
                            stop=(k == 7),
                        )
                    if t < 2:
                        nc.scalar.copy(k_bf[:, t * 512:(t + 1) * 512], ps)
                    else:
                        nc.vector.tensor_mul(
                            vg[:, (t - 2) * 512:(t - 1) * 512],
                            ps,
                            gts[c4][:, (t - 2) * 512:(t - 1) * 512],
                        )
                return k_bf, vg

            def emit_ctx(c4, kv_tiles):
                k_bf, vg = kv_tiles
                first = (blk == 0 and c4 == 0)
                last = (blk == NBLK - 1 and c4 == 3)
                for hc in range(8):
                    for hp in range(2):
                        h = hp * 8 + hc
                        dst = ctx_psA if hp == 0 else ctx_psB
                        # start=True clears has_written for the whole bank ->
                        # exactly one clearing matmul per bank.
                        nc.tensor.matmul(
                            dst[hp * 64:(hp + 1) * 64, hc * 64:(hc + 1) * 64],
                            vg[:, h * D:(h + 1) * D],
                            k_bf[:, h * D:(h + 1) * D],
                            start=(first and hc == 0),
                            stop=last,
                            skip_group_check=True,
                            tile_position=(0, hp * 64),
                        )

            kvt = {}
            kvt[0] = emit_kv(0)
            kvt[1] = emit_kv(1)
            emit_ctx(0, kvt[0])
            kvt[2] = emit_kv(2)
            emit_ctx(1, kvt[1])
            kvt[3] = emit_kv(3)
            emit_ctx(2, kvt[2])
            emit_ctx(3, kvt[3])

            if 2 <= blk <= 4:  # spread xq transposes through phase A
                emit_xq_trans(blk)

        # covers part of the softmax serial-chain bubble on PE
        emit_xq_trans(5)
        xq_tiles[7] = x_dma(xqin_pool, xq, 7, "xqin")

        # =========================================================
        # Softmax over d (free dim of ctxT) + build block-diag S pairs
        # st layout: heads 0-7 on partitions 0-63, heads 8-15 on 64-127.
        # =========================================================
        with ExitStack() as sm:
            smp = sm.enter_context(tc.tile_pool(name="smpool", bufs=1))
            maxs = smp.tile([128, 8], F32, name="maxs")
            cmx = smp.tile([128, 512], F32, name="cmx")
            sums = smp.tile([128, 8], F32, name="sums")
            et = mmps([128, 512], F32)
            halves = [(ctx_psA, slice(0, 64)), (ctx_psB, slice(64, 128))]
            for cps, sl in halves:
                nc.vector.tensor_reduce(
                    maxs[sl, :],
                    cps[sl, :].rearrange("p (b d) -> p b d", b=8),
                    axis=mybir.AxisListType.X,
                    op=mybir.AluOpType.max,
                )
                nc.vector.tensor_sub(
                    cmx[sl, :].rearrange("p (h d) -> p h d", h=8),
                    cps[sl, :].rearrange("p (h d) -> p h d", h=8),
                    maxs[sl, :].unsqueeze(-1).broadcast_to([64, 8, 64]),
                )
                nc.scalar.activation(
                    et[sl, :], cmx[sl, :], AF.Exp, scale=float(SCALE)
                )
            nc.vector.tensor_reduce(
                sums,
                et.rearrange("p (b d) -> p b d", b=8),
                axis=mybir.AxisListType.X,
                op=mybir.AluOpType.add,
            )
            recs = smp.tile([128, 8], F32, name="recs")
            nc.vector.reciprocal(recs, sums)
            st = smp.tile([128, 512], F32, name="st")
            nc.vector.tensor_mul(
                st.rearrange("p (h d) -> p h d", h=8),
                et.rearrange("p (h d) -> p h d", h=8),
                recs.unsqueeze(-1).broadcast_to([128, 8, 64]),
            )
            # Transposing the pair [ctxT_2j | ctxT_2j+1] ([64, 128]) gives
            # [S_2j stacked above S_2j+1] ([128, 64]); scatter to block-diag.
            for j in range(8):
                sl = slice(0, 64) if j < 4 else slice(64, 128)
                col = (2 * j) * 64 % 512
                tp = mmps([128, 64], F32)
                nc.tensor.transpose(
                    tp, st[sl, col:col + 128], ident2_sb[sl, :]
                )
                if j % 2 == 0:
                    nc.vector.tensor_copy(spairs[j][0:64, 0:64], tp[0:64, :])
                    nc.scalar.copy(spairs[j][64:128, 64:128], tp[64:128, :])
                else:
                    nc.scalar.copy(spairs[j][0:64, 0:64], tp[0:64, :])
                    nc.vector.tensor_copy(spairs[j][64:128, 64:128], tp[64:128, :])

        # =========================================================
        # Phase B: o[nchunk, j*128:(j+1)*128] = (xqT_j_chunk).T @ spair_j
        # =========================================================
        def emit_b(blk):
            xqT = xqT_tiles.pop(blk)
            for ch in range(2):  # half-block output granularity
                oout = oout_pool.tile([128, 2, C], BF16, name="oo", tag="oo")
                for cc in range(2):
                    c4 = ch * 2 + cc
                    for half in range(2):
                        ps = mmps([128, 512], F32)
                        for jj in range(4):
                            j = half * 4 + jj
                            nc.tensor.matmul(
                                ps[:, jj * 128:(jj + 1) * 128],
                                xqT[:, j, c4 * 128:(c4 + 1) * 128],
                                spairs[j],
                                start=True,
                                stop=True,
                                skip_group_check=True,
                            )
                        if half == 0:
                            nc.vector.tensor_copy(oout[:, cc, 0:512], ps)
                        else:
                            nc.scalar.copy(oout[:, cc, 512:1024], ps)
                nc.sync.dma_start(
                    o[blk * R + ch * 256:blk * R + (ch + 1) * 256, :].rearrange(
                        "(c p) m -> p c m", p=128
                    ),
                    oout,
                )

        emit_b(0)
        emit_xq_trans(6)
        emit_b(1)
        emit_xq_trans(7)
        for blk in range(2, NBLK):
            emit_b(blk)

    nc.compile()
    return nc


def _get_program(with_bias=False):
    key = ("nc", bool(with_bias))
    if key not in _CACHE:
        _CACHE[key] = _build_program(with_bias)
    return _CACHE[key]


def make_in_maps(x1, x2, Wkv1, Wkv2, g1_w1, g1_b1, g1_w2, g1_b2,
                 g2_w1, g2_b1, g2_w2, g2_b2):
    """Core (s, b): cores 0-3 = (s=0, b), cores 4-7 = (s=1, b)."""
    import ml_dtypes
    BF = ml_dtypes.bfloat16
    identb = np.eye(128, dtype=BF)
    eye64 = np.eye(64, dtype=np.float32)
    ident2 = np.ascontiguousarray(np.concatenate([eye64, eye64], axis=0))

    def bf(a):
        return np.ascontiguousarray(np.asarray(a, np.float32).astype(BF))

    x1b = [bf(x1[b]) for b in range(x1.shape[0])]
    x2b = [bf(x2[b]) for b in range(x2.shape[0])]
    Ws = [
        dict(wkv=bf(Wkv1), w1=bf(g1_w1), b1=np.asarray(g1_b1, np.float32),
             w2=bf(g1_w2), b2=bf(g1_b2)),
        dict(wkv=bf(Wkv2), w1=bf(g2_w1), b1=np.asarray(g2_b1, np.float32),
             w2=bf(g2_w2), b2=bf(g2_b2)),
    ]
    in_maps = []
    for core in range(8):
        s, b = core // 4, core % 4
        m = dict(Ws[s])
        m["xp"] = x1b[b] if s == 0 else x2b[b]
        m["xq"] = x2b[b] if s == 0 else x1b[b]
        m["identb"] = identb
        m["ident2"] = ident2
        in_maps.append(m)
    return in_maps


def kernel(x1, x2, Wkv1, Wkv2, g1_w1, g1_b1, g1_w2, g1_b2,
           g2_w1, g2_b1, g2_w2, g2_b2, _runner=None):
    """Full-input entry point.  Returns (o1, o2), each [4, 4096, 1024] f32."""
    from concourse.bass_utils import run_bass_kernel_spmd

    args = [np.asarray(a, dtype=np.float32) for a in
            (x1, x2, Wkv1, Wkv2, g1_w1, g1_b1, g1_w2, g1_b2,
             g2_w1, g2_b1, g2_w2, g2_b2)]
    with_bias = bool(np.any(args[7]) or np.any(args[11]))  # g1_b2, g2_b2
    nc = _get_program(with_bias)
    in_maps = make_in_maps(*args)
    if _runner is None:
        res = run_bass_kernel_spmd(nc, in_maps, core_ids=list(range(8)))
        results = res.results
    else:
        results = _runner(nc, in_maps)

    B = x1.shape[0]
    o1 = np.empty((B, N, C), dtype=np.float32)
    o2 = np.empty((B, N, C), dtype=np.float32)
    for core in range(8):
        s, b = core // 4, core % 4
        out = np.asarray(results[core]["o"], dtype=np.float32)
        if s == 0:
            o2[b] = out   # core projected x1 -> ctx1 -> o2 = q2 @ ctx1
        else:
            o1[b] = out
    return (o1, o2)
